# revision 29
# baseline (speedup 1.0000x reference)
"""Trainium2 Bass kernel for nn_LmLSTM: embedding -> 2x masked LSTM -> vocab projection.

Sharding: gate-sharded recurrence (core r owns hidden slice [128r,128r+128) of
both layers); full hidden state reassembled each step via AllGather of bf16
h-shards. The batch (B=16) is split into two halves that run as two
software-pipelined recurrences offset by half a step, so each half's
AllGather + sync latency hides behind the other half's compute.

The [H,V] output projection is vocab-sharded (4000 cols/core); tokens are laid
out (t, b)-major so projection chunks complete throughout the recurrence and
their GEMM work fills the PE idle time inside the recurrence loop. The whole
projection epilogue (bias, mask, onehot-for-masked-rows) is folded into the
PSUM accumulation via K=1 matmuls, and logits DMA straight from PSUM.

Per-step cell math for both layers is fused: one PSUM tile holds all 8 gate
groups [g1 g0 i1 i0 f1 f0 o1 o0] (biases pre-accumulated via K=1 matmuls), and
the elementwise chain runs on [128, 2*SB] combined tiles.
"""

import os
import sys
import types

import numpy as np
import ml_dtypes

# ---------------------------------------------------------------------------
# Environment shims (self-contained): NTFF profile hook + walrus wait-split.
# ---------------------------------------------------------------------------


def _install_axon_profile_hook():
    if "antenv.axon_hooks" in sys.modules:
        return
    holder = [None]
    mod = types.ModuleType("antenv.axon_hooks")
    mod.set_axon_ntff_profile_hook = lambda h: holder.__setitem__(0, h)
    mod.get_axon_ntff_profile_hook = lambda: holder[0]
    sys.modules["antenv.axon_hooks"] = mod
    try:
        import antenv

        antenv.axon_hooks = mod
        from trn_agent_boot.trn_boot import _ntff_profile_via_ctypes

        mod.set_axon_ntff_profile_hook(
            _ntff_profile_via_ctypes("/opt/axon/libaxon_pjrt.so")
        )
    except Exception:
        pass


_install_axon_profile_hook()

import concourse.bass as bass  # noqa: E402
import concourse.mybir as mybir  # noqa: E402
import concourse.tile as tile  # noqa: E402
from concourse.bass_utils import run_bass_kernel_spmd  # noqa: E402


def _install_wait_split():
    """This container's walrus accepts at most one sem-wait per instruction.
    Hoist excess waits onto same-engine nops placed just before."""
    if getattr(bass.Bass, "_waitsplit_installed", False):
        return
    counter = [0]

    def _split(m):
        for f in m.functions:
            for bb in f.blocks:
                il = bb.instructions
                if not any(
                    i.sync_info is not None and len(i.sync_info.on_wait) > 1
                    for i in il
                ):
                    continue
                new = []
                for inst in il:
                    si = inst.sync_info
                    if si is not None and len(si.on_wait) > 1:
                        waits = list(si.on_wait)
                        si.on_wait = waits[:1]
                        for w in waits[1:]:
                            counter[0] += 1
                            nop = mybir.InstNoOp(
                                name=f"waitsplit_{counter[0]}", ins=[], outs=[]
                            )
                            nop.engine = inst.engine
                            nop.sync_info = mybir.SyncInfo(
                                on_wait=[w], on_update=[]
                            )
                            new.append(nop)
                    new.append(inst)
                il.clear()
                il.extend(new)

    orig = bass.Bass.to_json_bytes

    def patched(self, *a, **kw):
        _split(self.m)
        return orig(self, *a, **kw)

    bass.Bass.to_json_bytes = patched
    bass.Bass._waitsplit_installed = True


_install_wait_split()

# ---------------------------------------------------------------------------
# Problem constants
# ---------------------------------------------------------------------------
V, E, H = 32000, 512, 1024
B = 16
T = int(os.environ.get("KERNEL_T", "256"))
NC = 8
VS = V // NC  # 4000 vocab cols per core
NTOK = B * T
NTC = NTOK // 128  # token chunks (8 t-steps x 16 b each)
SB = 8  # sub-batch width (two pipelined halves)
NSZ = VS // 8  # 500 vocab cols per projection n-group
F32 = mybir.dt.float32
BF16 = mybir.dt.bfloat16
U8 = mybir.dt.uint8
SIG = mybir.ActivationFunctionType.Sigmoid
TANH = mybir.ActivationFunctionType.Tanh

# psum gate-group layout (col = group*SB): [g1 g0 i1 i0 f1 f0 o1 o0]
G_G1, G_G0, G_I1, G_I0, G_F1, G_F0, G_O1, G_O0 = range(8)


def build_nc():
    nc = bass.Bass()
    d_w0 = nc.dram_tensor("w0p", [128, 12 * 4 * 128], BF16, kind="ExternalInput")
    d_w1 = nc.dram_tensor("w1p", [128, 16 * 4 * 128], BF16, kind="ExternalInput")
    d_wout = nc.dram_tensor("woutp", [128, 8 * VS], BF16, kind="ExternalInput")
    d_brow = nc.dram_tensor("brow", [1, 8 * 128], BF16, kind="ExternalInput")
    d_ones = nc.dram_tensor("onesr", [1, 128], BF16, kind="ExternalInput")
    d_boutr = nc.dram_tensor("boutr", [1, VS], BF16, kind="ExternalInput")
    d_ohcorr = nc.dram_tensor("ohcorr", [1, VS], BF16, kind="ExternalInput")
    d_imptb = nc.dram_tensor("imptb", [1, NTOK], BF16, kind="ExternalInput")
    d_xt = nc.dram_tensor("xt", [E, T * 16], BF16, kind="ExternalInput")
    d_masku = nc.dram_tensor("masku", [128, T * 16], U8, kind="ExternalInput")
    d_maskf = nc.dram_tensor("maskf", [128, T * 16], BF16, kind="ExternalInput")
    d_ident = nc.dram_tensor("ident", [128, 128], F32, kind="ExternalInput")
    d_identb = nc.dram_tensor("identb", [128, 128], BF16, kind="ExternalInput")
    d_out = nc.dram_tensor("out", [NTOK, VS], F32, kind="ExternalOutput")

    rg = [list(range(NC))]

    with tile.TileContext(nc) as tc:
        with (
            tc.tile_pool(name="wp", bufs=1) as wp,
            tc.tile_pool(name="sp", bufs=4) as sp,
            tc.tile_pool(name="pp", bufs=2, space="PSUM") as pp,
            tc.tile_pool(name="qq", bufs=2, space="PSUM") as qq,
            tc.tile_pool(name="dp", bufs=8, space="DRAM") as dp,
        ):
            # ---- persistent loads ----
            w0t = wp.tile([128, 12 * 4 * 128], BF16, tag="w0t")
            w1t = wp.tile([128, 16 * 4 * 128], BF16, tag="w1t")
            woutt = wp.tile([128, 8 * VS], BF16, tag="woutt")
            browt = wp.tile([1, 8 * 128], BF16, tag="browt")
            onest = wp.tile([1, 128], BF16, tag="onest")
            boutrt = wp.tile([1, VS], BF16, tag="boutrt")
            ohct = wp.tile([1, VS], BF16, tag="ohct")
            imptt = wp.tile([1, NTOK], BF16, tag="imptt")
            masku = wp.tile([128, T * 16], U8, tag="masku")
            maskf = wp.tile([128, T * 16], BF16, tag="maskf")
            ident = wp.tile([128, 128], F32, tag="ident")
            identb = wp.tile([128, 128], BF16, tag="identb")
            nc.gpsimd.dma_start(ident[:], d_ident[:])
            nc.gpsimd.dma_start(identb[:], d_identb[:])
            nc.gpsimd.dma_start(w0t[:], d_w0[:])
            nc.gpsimd.dma_start(w1t[:], d_w1[:])
            nc.gpsimd.dma_start(browt[:], d_brow[:])
            nc.gpsimd.dma_start(onest[:], d_ones[:])
            nc.gpsimd.dma_start(boutrt[:], d_boutr[:])
            nc.gpsimd.dma_start(ohct[:], d_ohcorr[:])
            nc.gpsimd.dma_start(imptt[:], d_imptb[:])
            nc.gpsimd.dma_start(masku[:], d_masku[:])
            nc.gpsimd.dma_start(maskf[:], d_maskf[:])
            nc.sync.dma_start(woutt[:], d_wout[:])
            xt = []
            for k in range(4):
                xk = wp.tile([128, T * 16], BF16, tag=f"xt{k}")
                nc.gpsimd.dma_start(xk[:], d_xt[128 * k : 128 * (k + 1), :])
                xt.append(xk)

            # stage ping-pong buffers: pre-masked h1 history for one proj
            # chunk, layout [128, (k 8)(j 8)(b 16)]
            stage0 = wp.tile([128, 8 * 8 * 16], BF16, tag="stage0")
            stage1 = wp.tile([128, 8 * 8 * 16], BF16, tag="stage1")
            stages = [stage0, stage1]

            # per-half persistent state: [c1 c0 h1 h0] f32
            state_a = wp.tile([128, 4 * SB], F32, tag="state_a")
            state_b = wp.tile([128, 4 * SB], F32, tag="state_b")
            state = [state_a, state_b]
            nc.vector.memset(state_a[:], 0.0)
            nc.vector.memset(state_b[:], 0.0)

            masuv = masku.rearrange("p (t b) -> p t b", b=16)
            masfv = maskf.rearrange("p (t b) -> p t b", b=16)

            # ---------------- projection -------------------------------
            def emit_proj(tcn, n):
                ps = qq.tile([128, NSZ], F32, tag="proj", bufs=1)
                stg = stages[tcn % 2]
                # bias row: ones (x) bout_n
                nc.tensor.matmul(
                    ps[:],
                    onest[:1, :],
                    boutrt[:1, n * NSZ : (n + 1) * NSZ],
                    start=True,
                    stop=False,
                )
                # masked-row correction: impt_tc (x) (onehot0 - bout)_n
                nc.tensor.matmul(
                    ps[:],
                    imptt[:1, 128 * tcn : 128 * (tcn + 1)],
                    ohct[:1, n * NSZ : (n + 1) * NSZ],
                    start=False,
                    stop=False,
                )
                for k in range(8):
                    nc.tensor.matmul(
                        ps[:],
                        stg[:, 128 * k : 128 * (k + 1)],
                        woutt[:, k * VS + n * NSZ : k * VS + (n + 1) * NSZ],
                        start=False,
                        stop=(k == 7),
                    )
                lg = sp.tile([128, NSZ], F32, tag="lg")
                nc.vector.tensor_copy(lg[:], ps[:])
                nc.sync.dma_start(
                    d_out[128 * tcn : 128 * (tcn + 1), n * NSZ : (n + 1) * NSZ],
                    lg[:],
                )

            # proj (tcn, n) emitted at iteration 8*tcn + 10 + n
            proj_sched = {}
            for tcn in range(NTC):
                for n in range(8):
                    proj_sched.setdefault(8 * tcn + 10 + n, []).append((tcn, n))
            proj_done = set()

            # ---------------- recurrence ---------------------------------
            # iteration n computes h0(n) (n<T) and h1(n-1) (n>=1), then ships
            # AG(n) = {h1(n-1), h0(n)}.  hfull from AG(n-1) holds blocks
            # [h1_k(n-2) at 2k, h0_k(n-1) at 2k+1].
            cc_outs = [[None] * 2 for _ in range(T + 1)]

            def stage_h1(hfull, j, h):
                stg = stages[(j // 8) % 2]
                src = hfull.rearrange("p (k l b) -> p k l b", k=8, l=2)[
                    :, :, 0, :
                ]
                dst = stg.rearrange("p (k j b) -> p k j b", k=8, j=8)[
                    :, :, j % 8, 8 * h : 8 * h + 8
                ]
                mkv = masfv[:, j : j + 1, 8 * h : 8 * h + 8].broadcast_to(
                    (128, 8, 8)
                )
                nc.vector.tensor_mul(dst, src, mkv)

            def receive(cco, h):
                # contiguous DMA of the AG output, then PE-transpose to land
                # it feature-major: hfull[p, k*16 + l*8 + b], l=0 h1, l=1 h0
                raw = sp.tile([128, 16 * SB], BF16, tag=f"raw{h}")
                nc.sync.dma_start(raw[:], cco[:])
                pt = pp.tile([128, 16 * SB], BF16, tag=f"pt{h}", bufs=1)
                nc.tensor.transpose(pt[:], raw[:], identb[:])
                hfull = sp.tile([128, 16 * SB], BF16, tag=f"hfull{h}")
                nc.vector.tensor_copy(hfull[:], pt[:])
                return hfull

            def emit_substep(n, h):
                # ---- receive AG(n-1) ----
                hfull = None
                if n >= 1:
                    hfull = receive(cc_outs[n - 1][h], h)
                    if n >= 2:
                        stage_h1(hfull, n - 2, h)

                # ---- gate matmuls into one PSUM tile ----
                zps = pp.tile([128, 8 * SB], F32, tag=f"z{h}")

                def gate_group(g, wtile, wbase, rhss):
                    dst = zps[:, g * SB : (g + 1) * SB]
                    nc.tensor.matmul(
                        dst,
                        browt[:, g * 128 : (g + 1) * 128],
                        onest[:1, 0:SB],
                        start=True,
                        stop=False,
                    )
                    nk = len(rhss)
                    for k, rhs in enumerate(rhss):
                        nc.tensor.matmul(
                            dst,
                            wtile[
                                :, (wbase + k * 4) * 128 : (wbase + k * 4) * 128 + 128
                            ],
                            rhs,
                            start=False,
                            stop=(k == nk - 1),
                        )

                l0 = n < T
                l1 = n >= 1
                if l1:
                    h0s = [hfull[:, 16 * k + 8 : 16 * k + 16] for k in range(8)]
                    h1s = [hfull[:, 16 * k : 16 * k + 8] for k in range(8)]
                    rhs1 = h0s + h1s
                if l0:
                    rhs0 = [
                        xk[:, n * 16 + 8 * h : n * 16 + 8 * h + 8] for xk in xt
                    ] + (
                        [hfull[:, 16 * k + 8 : 16 * k + 16] for k in range(8)]
                        if n >= 1
                        else []
                    )
                # gate order: g first (starts the chain), then i, f, o
                # gi is the gate's index in the packed weight layout [g,i,f,o]
                for gname, gi in (("g", 0), ("i", 1), ("f", 2), ("o", 3)):
                    if l1:
                        g1 = {"g": G_G1, "i": G_I1, "f": G_F1, "o": G_O1}[gname]
                        gate_group(g1, w1t, gi, rhs1)
                    if l0 and not (n == 0 and gname == "f"):
                        g0 = {"g": G_G0, "i": G_I0, "f": G_F0, "o": G_O0}[gname]
                        gate_group(g0, w0t, gi, rhs0)

                # ---- fused cell math ----
                # state/nt layout: [c1 c0 h1 h0] (c block and h block each
                # contiguous [128, 2*SB] in layer order l1, l0)
                gt = sp.tile([128, 8 * SB], F32, tag=f"gt{h}")
                nt = sp.tile([128, 4 * SB], F32, tag=f"nt{h}")
                st = state[h]
                CB, HB = slice(0, 2 * SB), slice(2 * SB, 4 * SB)

                def lv(ap):  # view a [128, 2*SB] slice as [128, l=2, b]
                    return ap.rearrange("p (l b) -> p l b", l=2)

                if l0 and l1:
                    nc.scalar.activation(gt[:, 0:16], zps[:, 0:16], TANH)
                    nc.scalar.activation(gt[:, 16:32], zps[:, 16:32], SIG)
                    nc.scalar.activation(gt[:, 32:48], zps[:, 32:48], SIG)
                    nc.scalar.activation(gt[:, 48:64], zps[:, 48:64], SIG)
                    tmpa = sp.tile([128, 2 * SB], F32, tag=f"tmpa{h}")
                    tmpb = sp.tile([128, 2 * SB], F32, tag=f"tmpb{h}")
                    tcn_ = sp.tile([128, 2 * SB], F32, tag=f"tcn{h}")
                    nc.vector.tensor_mul(tmpa[:], gt[:, 16:32], gt[:, 0:16])
                    nc.vector.tensor_mul(tmpb[:], gt[:, 32:48], st[:, CB])
                    nc.vector.tensor_add(nt[:, CB], tmpa[:], tmpb[:])
                    nc.scalar.activation(tcn_[:], nt[:, CB], TANH)
                    nc.vector.tensor_mul(nt[:, HB], gt[:, 48:64], tcn_[:])
                    mv = (
                        masuv[:, n - 1 : n + 1, 8 * h : 8 * h + 8]
                        .unsqueeze(1)
                        .broadcast_to((128, 2, 2, 8))
                    )
                    nc.vector.copy_predicated(
                        st.rearrange("p (r s b) -> p r s b", r=2, s=2),
                        mv,
                        nt.rearrange("p (r s b) -> p r s b", r=2, s=2),
                    )
                elif l0:
                    # n == 0: layer0 only, c=0 so cn = i*g
                    nc.scalar.activation(gt[:, 8:16], zps[:, 8:16], TANH)
                    nc.scalar.activation(gt[:, 24:32], zps[:, 24:32], SIG)
                    nc.scalar.activation(gt[:, 56:64], zps[:, 56:64], SIG)
                    tc0 = sp.tile([128, SB], F32, tag=f"tc0{h}")
                    nc.vector.tensor_mul(nt[:, SB : 2 * SB], gt[:, 24:32], gt[:, 8:16])
                    nc.scalar.activation(tc0[:], nt[:, SB : 2 * SB], TANH)
                    nc.vector.tensor_mul(nt[:, 3 * SB : 4 * SB], gt[:, 56:64], tc0[:])
                    mv = masuv[:, n : n + 1, 8 * h : 8 * h + 8].broadcast_to(
                        (128, 2, 8)
                    )
                    nc.vector.copy_predicated(
                        st.rearrange("p (r s b) -> p r s b", r=2, s=2)[:, :, 1, :],
                        mv,
                        nt.rearrange("p (r s b) -> p r s b", r=2, s=2)[:, :, 1, :],
                    )
                else:
                    # n == T: layer1 only
                    nc.scalar.activation(gt[:, 0:8], zps[:, 0:8], TANH)
                    nc.scalar.activation(gt[:, 16:24], zps[:, 16:24], SIG)
                    nc.scalar.activation(gt[:, 32:40], zps[:, 32:40], SIG)
                    nc.scalar.activation(gt[:, 48:56], zps[:, 48:56], SIG)
                    tmpa = sp.tile([128, SB], F32, tag=f"tmpa{h}")
                    tmpb = sp.tile([128, SB], F32, tag=f"tmpb{h}")
                    tc1 = sp.tile([128, SB], F32, tag=f"tc1{h}")
                    nc.vector.tensor_mul(tmpa[:], gt[:, 16:24], gt[:, 0:8])
                    nc.vector.tensor_mul(tmpb[:], gt[:, 32:40], st[:, 0:SB])
                    nc.vector.tensor_add(nt[:, 0:SB], tmpa[:], tmpb[:])
                    nc.scalar.activation(tc1[:], nt[:, 0:SB], TANH)
                    nc.vector.tensor_mul(nt[:, 2 * SB : 3 * SB], gt[:, 48:56], tc1[:])
                    mv = masuv[:, n - 1 : n, 8 * h : 8 * h + 8].broadcast_to(
                        (128, 2, 8)
                    )
                    nc.vector.copy_predicated(
                        st.rearrange("p (r s b) -> p r s b", r=2, s=2)[:, :, 0, :],
                        mv,
                        nt.rearrange("p (r s b) -> p r s b", r=2, s=2)[:, :, 0, :],
                    )

                # ---- ship AG(n) = {h1(n-1), h0(n)} ----
                # PE-transpose h to [16, 128] (feature-major rows), cast to
                # bf16, then a contiguous DMA feeds the AllGather.
                cc_in = dp.tile([2 * SB, 128], BF16, tag=f"cc_in{h}")
                cc_out = dp.tile([16 * SB, 128], BF16, tag=f"cc_out{h}")
                tp = pp.tile([2 * SB, 128], F32, tag="tp", bufs=1)
                nc.tensor.transpose(tp[:], st[:, HB], ident[:])
                hbt = sp.tile([2 * SB, 128], BF16, tag=f"hbt{h}")
                nc.scalar.activation(
                    hbt[:], tp[:], mybir.ActivationFunctionType.Copy
                )
                nc.scalar.dma_start(cc_in[:], hbt[:])
                nc.gpsimd.collective_compute(
                    "AllGather",
                    mybir.AluOpType.bypass,
                    ins=[cc_in.opt()],
                    outs=[cc_out.opt()],
                    replica_groups=rg,
                )
                cc_outs[n][h] = cc_out

            for n in range(T + 1):
                for tcn, pn in proj_sched.get(n, []):
                    emit_proj(tcn, pn)
                    proj_done.add((tcn, pn))
                for h in range(2):
                    emit_substep(n, h)

            # ---- epilogue: receive AG(T), stage h1(T-1), finish proj ----
            for h in range(2):
                hfullT = receive(cc_outs[T][h], h)
                stage_h1(hfullT, T - 1, h)
            for tcn in range(NTC):
                for n in range(8):
                    if (tcn, n) not in proj_done:
                        emit_proj(tcn, n)
    return nc


_NC_CACHE = [None]


def kernel(tokens, emb, Wx0, Wh0, b0, Wx1, Wh1, b1, Wout, bout):
    tokens = np.asarray(tokens)
    toks = tokens.astype(np.int64)
    emb = np.asarray(emb, np.float32)
    fm = (toks != 0).astype(np.float32)[:, :T]  # [B,T]

    x = emb[toks]  # [B,T,E]
    xt = np.ascontiguousarray(x[:, :T].transpose(2, 1, 0).reshape(E, T * B))
    xt = xt.astype(ml_dtypes.bfloat16)  # cols (t, b)

    fm_tb = np.ascontiguousarray(fm.T).reshape(-1)  # (t,b) order
    masku = np.broadcast_to(fm_tb.reshape(1, T * B), (128, T * B)).astype(np.uint8)
    maskf = masku.astype(ml_dtypes.bfloat16)
    imptb = (1.0 - fm_tb).reshape(1, T * B).astype(ml_dtypes.bfloat16)

    GO = [2, 0, 1, 3]  # gate order [g,i,f,o] from original (i,f,g,o)

    def pack(w, nk):
        # w: [nk*128, 512 cols in gate order] -> [128, nk*4*128]
        a = np.asarray(w, np.float32).reshape(nk, 128, 4, 128)
        return (
            np.ascontiguousarray(a.transpose(1, 0, 2, 3))
            .reshape(128, nk * 4 * 128)
            .astype(ml_dtypes.bfloat16)
        )

    ones = np.ones((1, 128), ml_dtypes.bfloat16)
    bouta = np.asarray(bout, np.float32)
    onehot0 = np.zeros((V,), np.float32)
    onehot0[0] = 1.0

    in_maps = []
    for r in range(NC):
        sl = np.arange(128 * r, 128 * (r + 1))
        cols = np.concatenate([g * H + sl for g in GO])
        w0 = np.concatenate([np.asarray(Wx0)[:, cols], np.asarray(Wh0)[:, cols]], 0)
        w1 = np.concatenate([np.asarray(Wx1)[:, cols], np.asarray(Wh1)[:, cols]], 0)
        wo = np.asarray(Wout, np.float32)[:, VS * r : VS * (r + 1)]  # [1024, VS]
        woutp = (
            np.ascontiguousarray(wo.reshape(8, 128, VS).transpose(1, 0, 2))
            .reshape(128, 8 * VS)
            .astype(ml_dtypes.bfloat16)
        )
        b0a = np.asarray(b0, np.float32)
        b1a = np.asarray(b1, np.float32)
        # bias row groups [g1 g0 i1 i0 f1 f0 o1 o0], original gates (i,f,g,o)
        brow = np.concatenate(
            [
                b1a[2 * H + sl], b0a[2 * H + sl],
                b1a[0 * H + sl], b0a[0 * H + sl],
                b1a[1 * H + sl], b0a[1 * H + sl],
                b1a[3 * H + sl], b0a[3 * H + sl],
            ]
        ).reshape(1, 8 * 128).astype(ml_dtypes.bfloat16)
        vsl = slice(VS * r, VS * (r + 1))
        in_maps.append(
            {
                "w0p": pack(w0, 12),
                "w1p": pack(w1, 16),
                "woutp": woutp,
                "brow": brow,
                "onesr": ones,
                "boutr": bouta[vsl].reshape(1, VS).astype(ml_dtypes.bfloat16),
                "ohcorr": (onehot0[vsl] - bouta[vsl])
                .reshape(1, VS)
                .astype(ml_dtypes.bfloat16),
                "imptb": imptb,
                "xt": xt,
                "masku": masku,
                "maskf": maskf,
                "ident": np.eye(128, dtype=np.float32),
                "identb": np.eye(128, dtype=ml_dtypes.bfloat16),
            }
        )

    if _NC_CACHE[0] is None:
        _NC_CACHE[0] = build_nc()
    nc = _NC_CACHE[0]

    trace = os.environ.get("KERNEL_TRACE", "0") == "1"
    res = run_bass_kernel_spmd(
        nc, in_maps, core_ids=list(range(NC)), trace=trace
    )
    if trace and res.exec_time_ns is not None:
        print(f"HW exec time: {res.exec_time_ns} ns")

    logits = np.concatenate(
        [res.results[r]["out"] for r in range(NC)], axis=1
    )  # [(t,b), V]
    out = np.ascontiguousarray(
        logits.reshape(T, B, V).transpose(1, 0, 2)
    ).astype(np.float32)
    if T < tokens.shape[1]:
        full = np.zeros((B, tokens.shape[1], V), np.float32)
        full[:, :T] = out
        out = full
    return out


# revision 30
# speedup vs baseline: 1.0105x; 1.0105x over previous
"""Trainium2 Bass kernel for nn_LmLSTM: embedding -> 2x masked LSTM -> vocab projection.

Sharding: gate-sharded recurrence (core r owns hidden slice [128r,128r+128) of
both layers); full hidden state reassembled each step via AllGather of bf16
h-shards. The batch (B=16) is split into two halves that run as two
software-pipelined recurrences offset by half a step, so each half's
AllGather + sync latency hides behind the other half's compute.

The [H,V] output projection is vocab-sharded (4000 cols/core); tokens are laid
out (t, b)-major so projection chunks complete throughout the recurrence and
their GEMM work fills the PE idle time inside the recurrence loop. The whole
projection epilogue (bias, mask, onehot-for-masked-rows) is folded into the
PSUM accumulation via K=1 matmuls, and logits DMA straight from PSUM.

Per-step cell math for both layers is fused: one PSUM tile holds all 8 gate
groups [g1 g0 i1 i0 f1 f0 o1 o0] (biases pre-accumulated via K=1 matmuls), and
the elementwise chain runs on [128, 2*SB] combined tiles.
"""

import os
import sys
import types

import numpy as np
import ml_dtypes

# ---------------------------------------------------------------------------
# Environment shims (self-contained): NTFF profile hook + walrus wait-split.
# ---------------------------------------------------------------------------


def _install_axon_profile_hook():
    if "antenv.axon_hooks" in sys.modules:
        return
    holder = [None]
    mod = types.ModuleType("antenv.axon_hooks")
    mod.set_axon_ntff_profile_hook = lambda h: holder.__setitem__(0, h)
    mod.get_axon_ntff_profile_hook = lambda: holder[0]
    sys.modules["antenv.axon_hooks"] = mod
    try:
        import antenv

        antenv.axon_hooks = mod
        from trn_agent_boot.trn_boot import _ntff_profile_via_ctypes

        mod.set_axon_ntff_profile_hook(
            _ntff_profile_via_ctypes("/opt/axon/libaxon_pjrt.so")
        )
    except Exception:
        pass


_install_axon_profile_hook()

import concourse.bass as bass  # noqa: E402
import concourse.mybir as mybir  # noqa: E402
import concourse.tile as tile  # noqa: E402
from concourse.bass_utils import run_bass_kernel_spmd  # noqa: E402


def _install_wait_split():
    """This container's walrus accepts at most one sem-wait per instruction.
    Hoist excess waits onto same-engine nops placed just before."""
    if getattr(bass.Bass, "_waitsplit_installed", False):
        return
    counter = [0]

    def _split(m):
        for f in m.functions:
            for bb in f.blocks:
                il = bb.instructions
                if not any(
                    i.sync_info is not None and len(i.sync_info.on_wait) > 1
                    for i in il
                ):
                    continue
                new = []
                for inst in il:
                    si = inst.sync_info
                    if si is not None and len(si.on_wait) > 1:
                        waits = list(si.on_wait)
                        si.on_wait = waits[:1]
                        for w in waits[1:]:
                            counter[0] += 1
                            nop = mybir.InstNoOp(
                                name=f"waitsplit_{counter[0]}", ins=[], outs=[]
                            )
                            nop.engine = inst.engine
                            nop.sync_info = mybir.SyncInfo(
                                on_wait=[w], on_update=[]
                            )
                            new.append(nop)
                    new.append(inst)
                il.clear()
                il.extend(new)

    orig = bass.Bass.to_json_bytes

    def patched(self, *a, **kw):
        _split(self.m)
        return orig(self, *a, **kw)

    bass.Bass.to_json_bytes = patched
    bass.Bass._waitsplit_installed = True


_install_wait_split()

# ---------------------------------------------------------------------------
# Problem constants
# ---------------------------------------------------------------------------
V, E, H = 32000, 512, 1024
B = 16
T = int(os.environ.get("KERNEL_T", "256"))
NC = 8
VS = V // NC  # 4000 vocab cols per core
NTOK = B * T
NTC = NTOK // 128  # token chunks (8 t-steps x 16 b each)
SB = 8  # sub-batch width (two pipelined halves)
NSZ = VS // 8  # 500 vocab cols per projection n-group
F32 = mybir.dt.float32
BF16 = mybir.dt.bfloat16
U8 = mybir.dt.uint8
SIG = mybir.ActivationFunctionType.Sigmoid
TANH = mybir.ActivationFunctionType.Tanh

# psum gate-group layout (col = group*SB): [g1 g0 i1 i0 f1 f0 o1 o0]
G_G1, G_G0, G_I1, G_I0, G_F1, G_F0, G_O1, G_O0 = range(8)


def build_nc():
    nc = bass.Bass()
    d_w0 = nc.dram_tensor("w0p", [128, 12 * 4 * 128], BF16, kind="ExternalInput")
    d_w1 = nc.dram_tensor("w1p", [128, 16 * 4 * 128], BF16, kind="ExternalInput")
    d_wout = nc.dram_tensor("woutp", [128, 8 * VS], BF16, kind="ExternalInput")
    d_brow = nc.dram_tensor("brow", [1, 8 * 128], BF16, kind="ExternalInput")
    d_ones = nc.dram_tensor("onesr", [1, 128], BF16, kind="ExternalInput")
    d_boutr = nc.dram_tensor("boutr", [1, VS], BF16, kind="ExternalInput")
    d_ohcorr = nc.dram_tensor("ohcorr", [1, VS], BF16, kind="ExternalInput")
    d_imptb = nc.dram_tensor("imptb", [1, NTOK], BF16, kind="ExternalInput")
    d_xt = nc.dram_tensor("xt", [E, T * 16], BF16, kind="ExternalInput")
    d_masku = nc.dram_tensor("masku", [128, T * 16], U8, kind="ExternalInput")
    d_maskf = nc.dram_tensor("maskf", [128, T * 16], BF16, kind="ExternalInput")
    d_ident = nc.dram_tensor("ident", [128, 128], F32, kind="ExternalInput")
    d_identb = nc.dram_tensor("identb", [128, 128], BF16, kind="ExternalInput")
    d_out = nc.dram_tensor("out", [NTOK, VS], F32, kind="ExternalOutput")

    rg = [list(range(NC))]

    with tile.TileContext(nc) as tc:
        with (
            tc.tile_pool(name="wp", bufs=1) as wp,
            tc.tile_pool(name="sp", bufs=4) as sp,
            tc.tile_pool(name="pp", bufs=2, space="PSUM") as pp,
            tc.tile_pool(name="qq", bufs=2, space="PSUM") as qq,
            tc.tile_pool(name="dp", bufs=8, space="DRAM") as dp,
        ):
            # ---- persistent loads ----
            w0t = wp.tile([128, 12 * 4 * 128], BF16, tag="w0t")
            w1t = wp.tile([128, 16 * 4 * 128], BF16, tag="w1t")
            woutt = wp.tile([128, 8 * VS], BF16, tag="woutt")
            browt = wp.tile([1, 8 * 128], BF16, tag="browt")
            onest = wp.tile([1, 128], BF16, tag="onest")
            boutrt = wp.tile([1, VS], BF16, tag="boutrt")
            ohct = wp.tile([1, VS], BF16, tag="ohct")
            imptt = wp.tile([1, NTOK], BF16, tag="imptt")
            masku = wp.tile([128, T * 16], U8, tag="masku")
            maskf = wp.tile([128, T * 16], BF16, tag="maskf")
            ident = wp.tile([128, 128], F32, tag="ident")
            identb = wp.tile([128, 128], BF16, tag="identb")
            nc.gpsimd.dma_start(ident[:], d_ident[:])
            nc.gpsimd.dma_start(identb[:], d_identb[:])
            nc.gpsimd.dma_start(w0t[:], d_w0[:])
            nc.gpsimd.dma_start(w1t[:], d_w1[:])
            nc.gpsimd.dma_start(browt[:], d_brow[:])
            nc.gpsimd.dma_start(onest[:], d_ones[:])
            nc.gpsimd.dma_start(boutrt[:], d_boutr[:])
            nc.gpsimd.dma_start(ohct[:], d_ohcorr[:])
            nc.gpsimd.dma_start(imptt[:], d_imptb[:])
            nc.gpsimd.dma_start(masku[:], d_masku[:])
            nc.gpsimd.dma_start(maskf[:], d_maskf[:])
            nc.sync.dma_start(woutt[:], d_wout[:])
            xt = []
            for k in range(4):
                xk = wp.tile([128, T * 16], BF16, tag=f"xt{k}")
                nc.gpsimd.dma_start(xk[:], d_xt[128 * k : 128 * (k + 1), :])
                xt.append(xk)

            # stage ping-pong buffers: pre-masked h1 history for one proj
            # chunk, layout [128, (k 8)(j 8)(b 16)]
            stage0 = wp.tile([128, 8 * 8 * 16], BF16, tag="stage0")
            stage1 = wp.tile([128, 8 * 8 * 16], BF16, tag="stage1")
            stages = [stage0, stage1]

            # per-half persistent state: [c1 c0 h1 h0] f32
            state_a = wp.tile([128, 4 * SB], F32, tag="state_a")
            state_b = wp.tile([128, 4 * SB], F32, tag="state_b")
            state = [state_a, state_b]
            nc.vector.memset(state_a[:], 0.0)
            nc.vector.memset(state_b[:], 0.0)

            masuv = masku.rearrange("p (t b) -> p t b", b=16)
            masfv = maskf.rearrange("p (t b) -> p t b", b=16)

            # ---------------- projection -------------------------------
            def emit_proj(tcn, n):
                ps = qq.tile([128, NSZ], F32, tag="proj", bufs=1)
                stg = stages[tcn % 2]
                # bias row: ones (x) bout_n
                nc.tensor.matmul(
                    ps[:],
                    onest[:1, :],
                    boutrt[:1, n * NSZ : (n + 1) * NSZ],
                    start=True,
                    stop=False,
                )
                # masked-row correction: impt_tc (x) (onehot0 - bout)_n
                nc.tensor.matmul(
                    ps[:],
                    imptt[:1, 128 * tcn : 128 * (tcn + 1)],
                    ohct[:1, n * NSZ : (n + 1) * NSZ],
                    start=False,
                    stop=False,
                )
                for k in range(8):
                    nc.tensor.matmul(
                        ps[:],
                        stg[:, 128 * k : 128 * (k + 1)],
                        woutt[:, k * VS + n * NSZ : k * VS + (n + 1) * NSZ],
                        start=False,
                        stop=(k == 7),
                    )
                lg = sp.tile([128, NSZ], F32, tag="lg")
                nc.vector.tensor_copy(lg[:], ps[:])
                nc.sync.dma_start(
                    d_out[128 * tcn : 128 * (tcn + 1), n * NSZ : (n + 1) * NSZ],
                    lg[:],
                )

            # proj (tcn, n) emitted at iteration 8*tcn + 10 + n
            proj_sched = {}
            for tcn in range(NTC):
                for n in range(8):
                    proj_sched.setdefault(8 * tcn + 10 + n, []).append((tcn, n))
            proj_done = set()

            # ---------------- recurrence ---------------------------------
            # iteration n computes h0(n) (n<T) and h1(n-1) (n>=1), then ships
            # AG(n) = {h1(n-1), h0(n)}.  hfull from AG(n-1) holds blocks
            # [h1_k(n-2) at 2k, h0_k(n-1) at 2k+1].
            cc_outs = [[None] * 2 for _ in range(T + 1)]

            def stage_h1(hfull, j, h):
                stg = stages[(j // 8) % 2]
                src = hfull.rearrange("p (k l b) -> p k l b", k=8, l=2)[
                    :, :, 0, :
                ]
                dst = stg.rearrange("p (k j b) -> p k j b", k=8, j=8)[
                    :, :, j % 8, 8 * h : 8 * h + 8
                ]
                mkv = masfv[:, j : j + 1, 8 * h : 8 * h + 8].broadcast_to(
                    (128, 8, 8)
                )
                nc.vector.tensor_mul(dst, src, mkv)

            def receive(cco, h):
                # contiguous DMA of the AG output, then PE-transpose to land
                # it feature-major: hfull[p, k*16 + l*8 + b], l=0 h1, l=1 h0
                raw = sp.tile([128, 16 * SB], BF16, tag=f"raw{h}")
                nc.sync.dma_start(raw[:], cco[:])
                pt = pp.tile([128, 16 * SB], BF16, tag=f"pt{h}", bufs=1)
                nc.tensor.transpose(pt[:], raw[:], identb[:])
                hfull = sp.tile([128, 16 * SB], BF16, tag=f"hfull{h}")
                nc.vector.tensor_copy(hfull[:], pt[:])
                return hfull

            def emit_substep(n, h):
                # ---- receive AG(n-1) ----
                hfull = None
                if n >= 1:
                    hfull = receive(cc_outs[n - 1][h], h)
                    if n >= 2:
                        stage_h1(hfull, n - 2, h)

                # ---- gate matmuls into one PSUM tile ----
                zps = pp.tile([128, 8 * SB], F32, tag=f"z{h}")

                def gate_group(g, wtile, wbase, rhss):
                    dst = zps[:, g * SB : (g + 1) * SB]
                    nc.tensor.matmul(
                        dst,
                        browt[:, g * 128 : (g + 1) * 128],
                        onest[:1, 0:SB],
                        start=True,
                        stop=False,
                    )
                    nk = len(rhss)
                    for k, rhs in enumerate(rhss):
                        nc.tensor.matmul(
                            dst,
                            wtile[
                                :, (wbase + k * 4) * 128 : (wbase + k * 4) * 128 + 128
                            ],
                            rhs,
                            start=False,
                            stop=(k == nk - 1),
                        )

                l0 = n < T
                l1 = n >= 1
                if l1:
                    h0s = [hfull[:, 16 * k + 8 : 16 * k + 16] for k in range(8)]
                    h1s = [hfull[:, 16 * k : 16 * k + 8] for k in range(8)]
                    rhs1 = h0s + h1s
                if l0:
                    rhs0 = [
                        xk[:, n * 16 + 8 * h : n * 16 + 8 * h + 8] for xk in xt
                    ] + (
                        [hfull[:, 16 * k + 8 : 16 * k + 16] for k in range(8)]
                        if n >= 1
                        else []
                    )
                # gate order: g first (starts the chain), then i, f, o
                # gi is the gate's index in the packed weight layout [g,i,f,o]
                for gname, gi in (("g", 0), ("i", 1), ("f", 2), ("o", 3)):
                    if l1:
                        g1 = {"g": G_G1, "i": G_I1, "f": G_F1, "o": G_O1}[gname]
                        gate_group(g1, w1t, gi, rhs1)
                    if l0 and not (n == 0 and gname == "f"):
                        g0 = {"g": G_G0, "i": G_I0, "f": G_F0, "o": G_O0}[gname]
                        gate_group(g0, w0t, gi, rhs0)

                # ---- fused cell math ----
                # state/nt layout: [c1 c0 h1 h0] (c block and h block each
                # contiguous [128, 2*SB] in layer order l1, l0)
                gt = sp.tile([128, 8 * SB], F32, tag=f"gt{h}")
                nt = sp.tile([128, 4 * SB], F32, tag=f"nt{h}")
                st = state[h]
                CB, HB = slice(0, 2 * SB), slice(2 * SB, 4 * SB)

                def lv(ap):  # view a [128, 2*SB] slice as [128, l=2, b]
                    return ap.rearrange("p (l b) -> p l b", l=2)

                if l0 and l1:
                    nc.scalar.activation(gt[:, 0:16], zps[:, 0:16], TANH)
                    nc.scalar.activation(gt[:, 16:32], zps[:, 16:32], SIG)
                    nc.scalar.activation(gt[:, 32:48], zps[:, 32:48], SIG)
                    nc.scalar.activation(gt[:, 48:64], zps[:, 48:64], SIG)
                    tmpa = sp.tile([128, 2 * SB], F32, tag=f"tmpa{h}")
                    tmpb = sp.tile([128, 2 * SB], F32, tag=f"tmpb{h}")
                    tcn_ = sp.tile([128, 2 * SB], F32, tag=f"tcn{h}")
                    nc.vector.tensor_mul(tmpa[:], gt[:, 16:32], gt[:, 0:16])
                    nc.vector.tensor_mul(tmpb[:], gt[:, 32:48], st[:, CB])
                    nc.vector.tensor_add(nt[:, CB], tmpa[:], tmpb[:])
                    nc.scalar.activation(tcn_[:], nt[:, CB], TANH)
                    nc.vector.tensor_mul(nt[:, HB], gt[:, 48:64], tcn_[:])
                    mv = (
                        masuv[:, n - 1 : n + 1, 8 * h : 8 * h + 8]
                        .unsqueeze(1)
                        .broadcast_to((128, 2, 2, 8))
                    )
                    nc.vector.copy_predicated(
                        st.rearrange("p (r s b) -> p r s b", r=2, s=2),
                        mv,
                        nt.rearrange("p (r s b) -> p r s b", r=2, s=2),
                    )
                elif l0:
                    # n == 0: layer0 only, c=0 so cn = i*g
                    nc.scalar.activation(gt[:, 8:16], zps[:, 8:16], TANH)
                    nc.scalar.activation(gt[:, 24:32], zps[:, 24:32], SIG)
                    nc.scalar.activation(gt[:, 56:64], zps[:, 56:64], SIG)
                    tc0 = sp.tile([128, SB], F32, tag=f"tc0{h}")
                    nc.vector.tensor_mul(nt[:, SB : 2 * SB], gt[:, 24:32], gt[:, 8:16])
                    nc.scalar.activation(tc0[:], nt[:, SB : 2 * SB], TANH)
                    nc.vector.tensor_mul(nt[:, 3 * SB : 4 * SB], gt[:, 56:64], tc0[:])
                    mv = masuv[:, n : n + 1, 8 * h : 8 * h + 8].broadcast_to(
                        (128, 2, 8)
                    )
                    nc.vector.copy_predicated(
                        st.rearrange("p (r s b) -> p r s b", r=2, s=2)[:, :, 1, :],
                        mv,
                        nt.rearrange("p (r s b) -> p r s b", r=2, s=2)[:, :, 1, :],
                    )
                else:
                    # n == T: layer1 only
                    nc.scalar.activation(gt[:, 0:8], zps[:, 0:8], TANH)
                    nc.scalar.activation(gt[:, 16:24], zps[:, 16:24], SIG)
                    nc.scalar.activation(gt[:, 32:40], zps[:, 32:40], SIG)
                    nc.scalar.activation(gt[:, 48:56], zps[:, 48:56], SIG)
                    tmpa = sp.tile([128, SB], F32, tag=f"tmpa{h}")
                    tmpb = sp.tile([128, SB], F32, tag=f"tmpb{h}")
                    tc1 = sp.tile([128, SB], F32, tag=f"tc1{h}")
                    nc.vector.tensor_mul(tmpa[:], gt[:, 16:24], gt[:, 0:8])
                    nc.vector.tensor_mul(tmpb[:], gt[:, 32:40], st[:, 0:SB])
                    nc.vector.tensor_add(nt[:, 0:SB], tmpa[:], tmpb[:])
                    nc.scalar.activation(tc1[:], nt[:, 0:SB], TANH)
                    nc.vector.tensor_mul(nt[:, 2 * SB : 3 * SB], gt[:, 48:56], tc1[:])
                    mv = masuv[:, n - 1 : n, 8 * h : 8 * h + 8].broadcast_to(
                        (128, 2, 8)
                    )
                    nc.vector.copy_predicated(
                        st.rearrange("p (r s b) -> p r s b", r=2, s=2)[:, :, 0, :],
                        mv,
                        nt.rearrange("p (r s b) -> p r s b", r=2, s=2)[:, :, 0, :],
                    )

            def emit_send(n, h):
                # ---- ship AG(n) = {h1(n-1), h0(n)} ----
                # PE-transpose h to [16, 128] (feature-major rows), cast to
                # bf16, then a contiguous DMA feeds the AllGather.  Emitted
                # after BOTH halves' compute phases so the transpose's PE-FIFO
                # stall never blocks the other half's gate matmuls.
                st = state[h]
                cc_in = dp.tile([2 * SB, 128], BF16, tag=f"cc_in{h}")
                cc_out = dp.tile([16 * SB, 128], BF16, tag=f"cc_out{h}")
                tp = pp.tile([2 * SB, 128], F32, tag="tp", bufs=1)
                nc.tensor.transpose(tp[:], st[:, 2 * SB : 4 * SB], ident[:])
                hbt = sp.tile([2 * SB, 128], BF16, tag=f"hbt{h}")
                nc.scalar.activation(
                    hbt[:], tp[:], mybir.ActivationFunctionType.Copy
                )
                nc.scalar.dma_start(cc_in[:], hbt[:])
                nc.gpsimd.collective_compute(
                    "AllGather",
                    mybir.AluOpType.bypass,
                    ins=[cc_in.opt()],
                    outs=[cc_out.opt()],
                    replica_groups=rg,
                )
                cc_outs[n][h] = cc_out

            for n in range(T + 1):
                for tcn, pn in proj_sched.get(n, []):
                    emit_proj(tcn, pn)
                    proj_done.add((tcn, pn))
                for h in range(2):
                    emit_substep(n, h)
                for h in range(2):
                    emit_send(n, h)

            # ---- epilogue: receive AG(T), stage h1(T-1), finish proj ----
            for h in range(2):
                hfullT = receive(cc_outs[T][h], h)
                stage_h1(hfullT, T - 1, h)
            for tcn in range(NTC):
                for n in range(8):
                    if (tcn, n) not in proj_done:
                        emit_proj(tcn, n)
    return nc


_NC_CACHE = [None]


def kernel(tokens, emb, Wx0, Wh0, b0, Wx1, Wh1, b1, Wout, bout):
    tokens = np.asarray(tokens)
    toks = tokens.astype(np.int64)
    emb = np.asarray(emb, np.float32)
    fm = (toks != 0).astype(np.float32)[:, :T]  # [B,T]

    x = emb[toks]  # [B,T,E]
    xt = np.ascontiguousarray(x[:, :T].transpose(2, 1, 0).reshape(E, T * B))
    xt = xt.astype(ml_dtypes.bfloat16)  # cols (t, b)

    fm_tb = np.ascontiguousarray(fm.T).reshape(-1)  # (t,b) order
    masku = np.broadcast_to(fm_tb.reshape(1, T * B), (128, T * B)).astype(np.uint8)
    maskf = masku.astype(ml_dtypes.bfloat16)
    imptb = (1.0 - fm_tb).reshape(1, T * B).astype(ml_dtypes.bfloat16)

    GO = [2, 0, 1, 3]  # gate order [g,i,f,o] from original (i,f,g,o)

    def pack(w, nk):
        # w: [nk*128, 512 cols in gate order] -> [128, nk*4*128]
        a = np.asarray(w, np.float32).reshape(nk, 128, 4, 128)
        return (
            np.ascontiguousarray(a.transpose(1, 0, 2, 3))
            .reshape(128, nk * 4 * 128)
            .astype(ml_dtypes.bfloat16)
        )

    ones = np.ones((1, 128), ml_dtypes.bfloat16)
    bouta = np.asarray(bout, np.float32)
    onehot0 = np.zeros((V,), np.float32)
    onehot0[0] = 1.0

    in_maps = []
    for r in range(NC):
        sl = np.arange(128 * r, 128 * (r + 1))
        cols = np.concatenate([g * H + sl for g in GO])
        w0 = np.concatenate([np.asarray(Wx0)[:, cols], np.asarray(Wh0)[:, cols]], 0)
        w1 = np.concatenate([np.asarray(Wx1)[:, cols], np.asarray(Wh1)[:, cols]], 0)
        wo = np.asarray(Wout, np.float32)[:, VS * r : VS * (r + 1)]  # [1024, VS]
        woutp = (
            np.ascontiguousarray(wo.reshape(8, 128, VS).transpose(1, 0, 2))
            .reshape(128, 8 * VS)
            .astype(ml_dtypes.bfloat16)
        )
        b0a = np.asarray(b0, np.float32)
        b1a = np.asarray(b1, np.float32)
        # bias row groups [g1 g0 i1 i0 f1 f0 o1 o0], original gates (i,f,g,o)
        brow = np.concatenate(
            [
                b1a[2 * H + sl], b0a[2 * H + sl],
                b1a[0 * H + sl], b0a[0 * H + sl],
                b1a[1 * H + sl], b0a[1 * H + sl],
                b1a[3 * H + sl], b0a[3 * H + sl],
            ]
        ).reshape(1, 8 * 128).astype(ml_dtypes.bfloat16)
        vsl = slice(VS * r, VS * (r + 1))
        in_maps.append(
            {
                "w0p": pack(w0, 12),
                "w1p": pack(w1, 16),
                "woutp": woutp,
                "brow": brow,
                "onesr": ones,
                "boutr": bouta[vsl].reshape(1, VS).astype(ml_dtypes.bfloat16),
                "ohcorr": (onehot0[vsl] - bouta[vsl])
                .reshape(1, VS)
                .astype(ml_dtypes.bfloat16),
                "imptb": imptb,
                "xt": xt,
                "masku": masku,
                "maskf": maskf,
                "ident": np.eye(128, dtype=np.float32),
                "identb": np.eye(128, dtype=ml_dtypes.bfloat16),
            }
        )

    if _NC_CACHE[0] is None:
        _NC_CACHE[0] = build_nc()
    nc = _NC_CACHE[0]

    trace = os.environ.get("KERNEL_TRACE", "0") == "1"
    res = run_bass_kernel_spmd(
        nc, in_maps, core_ids=list(range(NC)), trace=trace
    )
    if trace and res.exec_time_ns is not None:
        print(f"HW exec time: {res.exec_time_ns} ns")

    logits = np.concatenate(
        [res.results[r]["out"] for r in range(NC)], axis=1
    )  # [(t,b), V]
    out = np.ascontiguousarray(
        logits.reshape(T, B, V).transpose(1, 0, 2)
    ).astype(np.float32)
    if T < tokens.shape[1]:
        full = np.zeros((B, tokens.shape[1], V), np.float32)
        full[:, :T] = out
        out = full
    return out


# revision 32
# speedup vs baseline: 1.0606x; 1.0496x over previous
"""Trainium2 Bass kernel for nn_LmLSTM: embedding -> 2x masked LSTM -> vocab projection.

Sharding: gate-sharded recurrence (core r owns hidden slice [128r,128r+128) of
both layers); full hidden state reassembled each step via AllGather of bf16
h-shards. The batch (B=16) is split into two halves that run as two
software-pipelined recurrences offset by half a step, so each half's
AllGather + sync latency hides behind the other half's compute.

The [H,V] output projection is vocab-sharded (4000 cols/core); tokens are laid
out (t, b)-major so projection chunks complete throughout the recurrence and
their GEMM work fills the PE idle time inside the recurrence loop. The whole
projection epilogue (bias, mask, onehot-for-masked-rows) is folded into the
PSUM accumulation via K=1 matmuls, and logits DMA straight from PSUM.

Per-step cell math for both layers is fused: one PSUM tile holds all 8 gate
groups [g1 g0 i1 i0 f1 f0 o1 o0] (biases pre-accumulated via K=1 matmuls), and
the elementwise chain runs on [128, 2*SB] combined tiles.
"""

import os
import sys
import types

import numpy as np
import ml_dtypes

# ---------------------------------------------------------------------------
# Environment shims (self-contained): NTFF profile hook + walrus wait-split.
# ---------------------------------------------------------------------------


def _install_axon_profile_hook():
    if "antenv.axon_hooks" in sys.modules:
        return
    holder = [None]
    mod = types.ModuleType("antenv.axon_hooks")
    mod.set_axon_ntff_profile_hook = lambda h: holder.__setitem__(0, h)
    mod.get_axon_ntff_profile_hook = lambda: holder[0]
    sys.modules["antenv.axon_hooks"] = mod
    try:
        import antenv

        antenv.axon_hooks = mod
        from trn_agent_boot.trn_boot import _ntff_profile_via_ctypes

        mod.set_axon_ntff_profile_hook(
            _ntff_profile_via_ctypes("/opt/axon/libaxon_pjrt.so")
        )
    except Exception:
        pass


_install_axon_profile_hook()

import concourse.bass as bass  # noqa: E402
import concourse.mybir as mybir  # noqa: E402
import concourse.tile as tile  # noqa: E402
from concourse.bass_utils import run_bass_kernel_spmd  # noqa: E402


def _install_wait_split():
    """This container's walrus accepts at most one sem-wait per instruction.
    Hoist excess waits onto same-engine nops placed just before."""
    if getattr(bass.Bass, "_waitsplit_installed", False):
        return
    counter = [0]

    def _split(m):
        for f in m.functions:
            for bb in f.blocks:
                il = bb.instructions
                if not any(
                    i.sync_info is not None and len(i.sync_info.on_wait) > 1
                    for i in il
                ):
                    continue
                new = []
                for inst in il:
                    si = inst.sync_info
                    if si is not None and len(si.on_wait) > 1:
                        waits = list(si.on_wait)
                        si.on_wait = waits[:1]
                        for w in waits[1:]:
                            counter[0] += 1
                            nop = mybir.InstNoOp(
                                name=f"waitsplit_{counter[0]}", ins=[], outs=[]
                            )
                            nop.engine = inst.engine
                            nop.sync_info = mybir.SyncInfo(
                                on_wait=[w], on_update=[]
                            )
                            new.append(nop)
                    new.append(inst)
                il.clear()
                il.extend(new)

    orig = bass.Bass.to_json_bytes

    def patched(self, *a, **kw):
        _split(self.m)
        return orig(self, *a, **kw)

    bass.Bass.to_json_bytes = patched
    bass.Bass._waitsplit_installed = True


_install_wait_split()

# ---------------------------------------------------------------------------
# Problem constants
# ---------------------------------------------------------------------------
V, E, H = 32000, 512, 1024
B = 16
T = int(os.environ.get("KERNEL_T", "256"))
NC = 8
VS = V // NC  # 4000 vocab cols per core
NTOK = B * T
NTC = NTOK // 128  # token chunks (8 t-steps x 16 b each)
SB = 8  # sub-batch width (two pipelined halves)
NSZ = VS // 8  # 500 vocab cols per projection n-group
F32 = mybir.dt.float32
BF16 = mybir.dt.bfloat16
U8 = mybir.dt.uint8
SIG = mybir.ActivationFunctionType.Sigmoid
TANH = mybir.ActivationFunctionType.Tanh

# psum gate-group layout (col = group*SB): [g1 g0 i1 i0 f1 f0 o1 o0]
G_G1, G_G0, G_I1, G_I0, G_F1, G_F0, G_O1, G_O0 = range(8)


def build_nc():
    nc = bass.Bass()
    d_w0 = nc.dram_tensor("w0p", [128, 12 * 4 * 128], BF16, kind="ExternalInput")
    d_w1 = nc.dram_tensor("w1p", [128, 16 * 4 * 128], BF16, kind="ExternalInput")
    d_wout = nc.dram_tensor("woutp", [128, 8 * VS], BF16, kind="ExternalInput")
    d_brow = nc.dram_tensor("brow", [1, 8 * 128], BF16, kind="ExternalInput")
    d_ones = nc.dram_tensor("onesr", [1, 128], BF16, kind="ExternalInput")
    d_boutr = nc.dram_tensor("boutr", [1, VS], BF16, kind="ExternalInput")
    d_ohcorr = nc.dram_tensor("ohcorr", [1, VS], BF16, kind="ExternalInput")
    d_imptb = nc.dram_tensor("imptb", [1, NTOK], BF16, kind="ExternalInput")
    d_xt = nc.dram_tensor("xt", [E, T * 16], BF16, kind="ExternalInput")
    d_masku = nc.dram_tensor("masku", [128, T * 16], U8, kind="ExternalInput")
    d_maskf = nc.dram_tensor("maskf", [128, T * 16], BF16, kind="ExternalInput")
    d_ident = nc.dram_tensor("ident", [128, 128], F32, kind="ExternalInput")
    d_identb = nc.dram_tensor("identb", [128, 128], BF16, kind="ExternalInput")
    d_out = nc.dram_tensor("out", [NTOK, VS], F32, kind="ExternalOutput")

    rg = [list(range(NC))]

    with tile.TileContext(nc) as tc:
        with (
            tc.tile_pool(name="wp", bufs=1) as wp,
            tc.tile_pool(name="sp", bufs=4) as sp,
            tc.tile_pool(name="pp", bufs=2, space="PSUM") as pp,
            tc.tile_pool(name="qq", bufs=2, space="PSUM") as qq,
            tc.tile_pool(name="dp", bufs=8, space="DRAM") as dp,
        ):
            # ---- persistent loads ----
            w0t = wp.tile([128, 12 * 4 * 128], BF16, tag="w0t")
            w1t = wp.tile([128, 16 * 4 * 128], BF16, tag="w1t")
            woutt = wp.tile([128, 8 * VS], BF16, tag="woutt")
            browt = wp.tile([1, 8 * 128], BF16, tag="browt")
            onest = wp.tile([1, 128], BF16, tag="onest")
            boutrt = wp.tile([1, VS], BF16, tag="boutrt")
            ohct = wp.tile([1, VS], BF16, tag="ohct")
            imptt = wp.tile([1, NTOK], BF16, tag="imptt")
            masku = wp.tile([128, T * 16], U8, tag="masku")
            maskf = wp.tile([128, T * 16], BF16, tag="maskf")
            ident = wp.tile([128, 128], F32, tag="ident")
            identb = wp.tile([128, 128], BF16, tag="identb")
            nc.gpsimd.dma_start(ident[:], d_ident[:])
            nc.gpsimd.dma_start(identb[:], d_identb[:])
            nc.gpsimd.dma_start(w0t[:], d_w0[:])
            nc.gpsimd.dma_start(w1t[:], d_w1[:])
            nc.gpsimd.dma_start(browt[:], d_brow[:])
            nc.gpsimd.dma_start(onest[:], d_ones[:])
            nc.gpsimd.dma_start(boutrt[:], d_boutr[:])
            nc.gpsimd.dma_start(ohct[:], d_ohcorr[:])
            nc.gpsimd.dma_start(imptt[:], d_imptb[:])
            nc.gpsimd.dma_start(masku[:], d_masku[:])
            nc.gpsimd.dma_start(maskf[:], d_maskf[:])
            nc.sync.dma_start(woutt[:], d_wout[:])
            xt = []
            for k in range(4):
                xk = wp.tile([128, T * 16], BF16, tag=f"xt{k}")
                nc.gpsimd.dma_start(xk[:], d_xt[128 * k : 128 * (k + 1), :])
                xt.append(xk)

            # stage ping-pong buffers: pre-masked h1 history for one proj
            # chunk, layout [128, (k 8)(j 8)(b 16)]
            stage0 = wp.tile([128, 8 * 8 * 16], BF16, tag="stage0")
            stage1 = wp.tile([128, 8 * 8 * 16], BF16, tag="stage1")
            stages = [stage0, stage1]

            # per-half persistent state: [c1 c0 h1 h0] f32
            state_a = wp.tile([128, 4 * SB], F32, tag="state_a")
            state_b = wp.tile([128, 4 * SB], F32, tag="state_b")
            state = [state_a, state_b]
            nc.vector.memset(state_a[:], 0.0)
            nc.vector.memset(state_b[:], 0.0)

            masuv = masku.rearrange("p (t b) -> p t b", b=16)
            masfv = maskf.rearrange("p (t b) -> p t b", b=16)

            # ---------------- projection -------------------------------
            def emit_proj(tcn, n):
                ps = qq.tile([128, NSZ], F32, tag="proj", bufs=1)
                stg = stages[tcn % 2]
                # bias row: ones (x) bout_n
                nc.tensor.matmul(
                    ps[:],
                    onest[:1, :],
                    boutrt[:1, n * NSZ : (n + 1) * NSZ],
                    start=True,
                    stop=False,
                )
                # masked-row correction: impt_tc (x) (onehot0 - bout)_n
                nc.tensor.matmul(
                    ps[:],
                    imptt[:1, 128 * tcn : 128 * (tcn + 1)],
                    ohct[:1, n * NSZ : (n + 1) * NSZ],
                    start=False,
                    stop=False,
                )
                for k in range(8):
                    nc.tensor.matmul(
                        ps[:],
                        stg[:, 128 * k : 128 * (k + 1)],
                        woutt[:, k * VS + n * NSZ : k * VS + (n + 1) * NSZ],
                        start=False,
                        stop=(k == 7),
                    )
                lg = sp.tile([128, NSZ], F32, tag="lg")
                nc.vector.tensor_copy(lg[:], ps[:])
                nc.gpsimd.dma_start(
                    d_out[128 * tcn : 128 * (tcn + 1), n * NSZ : (n + 1) * NSZ],
                    lg[:],
                )

            # proj (tcn, n) emitted at iteration 8*tcn + 10 + n
            proj_sched = {}
            for tcn in range(NTC):
                for n in range(8):
                    proj_sched.setdefault(8 * tcn + 10 + n, []).append((tcn, n))
            proj_done = set()

            # ---------------- recurrence ---------------------------------
            # iteration n computes h0(n) (n<T) and h1(n-1) (n>=1), then ships
            # AG(n) = {h1(n-1), h0(n)}.  hfull from AG(n-1) holds blocks
            # [h1_k(n-2) at 2k, h0_k(n-1) at 2k+1].
            cc_outs = [[None] * 2 for _ in range(T + 1)]

            def stage_h1(hfull, j, h):
                stg = stages[(j // 8) % 2]
                src = hfull.rearrange("p (k l b) -> p k l b", k=8, l=2)[
                    :, :, 0, :
                ]
                dst = stg.rearrange("p (k j b) -> p k j b", k=8, j=8)[
                    :, :, j % 8, 8 * h : 8 * h + 8
                ]
                mkv = masfv[:, j : j + 1, 8 * h : 8 * h + 8].broadcast_to(
                    (128, 8, 8)
                )
                nc.vector.tensor_mul(dst, src, mkv)

            def receive(cco, h):
                # contiguous DMA of the AG output, then PE-transpose to land
                # it feature-major: hfull[p, k*16 + l*8 + b], l=0 h1, l=1 h0
                raw = sp.tile([128, 16 * SB], BF16, tag=f"raw{h}")
                nc.sync.dma_start(raw[:], cco[:])
                pt = pp.tile([128, 16 * SB], BF16, tag=f"pt{h}", bufs=1)
                nc.tensor.transpose(pt[:], raw[:], identb[:])
                hfull = sp.tile([128, 16 * SB], BF16, tag=f"hfull{h}")
                nc.vector.tensor_copy(hfull[:], pt[:])
                return hfull

            def emit_substep(n, h):
                # ---- receive AG(n-1) ----
                hfull = None
                if n >= 1:
                    hfull = receive(cc_outs[n - 1][h], h)
                    if n >= 2:
                        stage_h1(hfull, n - 2, h)

                # ---- gate matmuls into one PSUM tile ----
                zps = pp.tile([128, 8 * SB], F32, tag=f"z{h}")

                def gate_group(g, wtile, wbase, rhss):
                    dst = zps[:, g * SB : (g + 1) * SB]
                    nc.tensor.matmul(
                        dst,
                        browt[:, g * 128 : (g + 1) * 128],
                        onest[:1, 0:SB],
                        start=True,
                        stop=False,
                    )
                    nk = len(rhss)
                    for k, rhs in enumerate(rhss):
                        nc.tensor.matmul(
                            dst,
                            wtile[
                                :, (wbase + k * 4) * 128 : (wbase + k * 4) * 128 + 128
                            ],
                            rhs,
                            start=False,
                            stop=(k == nk - 1),
                        )

                l0 = n < T
                l1 = n >= 1
                if l1:
                    h0s = [hfull[:, 16 * k + 8 : 16 * k + 16] for k in range(8)]
                    h1s = [hfull[:, 16 * k : 16 * k + 8] for k in range(8)]
                    rhs1 = h0s + h1s
                if l0:
                    rhs0 = [
                        xk[:, n * 16 + 8 * h : n * 16 + 8 * h + 8] for xk in xt
                    ] + (
                        [hfull[:, 16 * k + 8 : 16 * k + 16] for k in range(8)]
                        if n >= 1
                        else []
                    )
                # gate order: g first (starts the chain), then i, f, o
                # gi is the gate's index in the packed weight layout [g,i,f,o]
                for gname, gi in (("g", 0), ("i", 1), ("f", 2), ("o", 3)):
                    if l1:
                        g1 = {"g": G_G1, "i": G_I1, "f": G_F1, "o": G_O1}[gname]
                        gate_group(g1, w1t, gi, rhs1)
                    if l0 and not (n == 0 and gname == "f"):
                        g0 = {"g": G_G0, "i": G_I0, "f": G_F0, "o": G_O0}[gname]
                        gate_group(g0, w0t, gi, rhs0)

                # ---- fused cell math ----
                # state/nt layout: [c1 c0 h1 h0] (c block and h block each
                # contiguous [128, 2*SB] in layer order l1, l0)
                gt = sp.tile([128, 8 * SB], F32, tag=f"gt{h}")
                nt = sp.tile([128, 4 * SB], F32, tag=f"nt{h}")
                st = state[h]
                CB, HB = slice(0, 2 * SB), slice(2 * SB, 4 * SB)

                def lv(ap):  # view a [128, 2*SB] slice as [128, l=2, b]
                    return ap.rearrange("p (l b) -> p l b", l=2)

                if l0 and l1:
                    nc.scalar.activation(gt[:, 0:16], zps[:, 0:16], TANH)
                    nc.scalar.activation(gt[:, 16:32], zps[:, 16:32], SIG)
                    nc.scalar.activation(gt[:, 32:48], zps[:, 32:48], SIG)
                    nc.scalar.activation(gt[:, 48:64], zps[:, 48:64], SIG)
                    tmpa = sp.tile([128, 2 * SB], F32, tag=f"tmpa{h}")
                    tmpb = sp.tile([128, 2 * SB], F32, tag=f"tmpb{h}")
                    tcn_ = sp.tile([128, 2 * SB], F32, tag=f"tcn{h}")
                    nc.vector.tensor_mul(tmpa[:], gt[:, 16:32], gt[:, 0:16])
                    nc.vector.tensor_mul(tmpb[:], gt[:, 32:48], st[:, CB])
                    nc.vector.tensor_add(nt[:, CB], tmpa[:], tmpb[:])
                    nc.scalar.activation(tcn_[:], nt[:, CB], TANH)
                    nc.vector.tensor_mul(nt[:, HB], gt[:, 48:64], tcn_[:])
                    mv = (
                        masuv[:, n - 1 : n + 1, 8 * h : 8 * h + 8]
                        .unsqueeze(1)
                        .broadcast_to((128, 2, 2, 8))
                    )
                    nc.vector.copy_predicated(
                        st.rearrange("p (r s b) -> p r s b", r=2, s=2),
                        mv,
                        nt.rearrange("p (r s b) -> p r s b", r=2, s=2),
                    )
                elif l0:
                    # n == 0: layer0 only, c=0 so cn = i*g
                    nc.scalar.activation(gt[:, 8:16], zps[:, 8:16], TANH)
                    nc.scalar.activation(gt[:, 24:32], zps[:, 24:32], SIG)
                    nc.scalar.activation(gt[:, 56:64], zps[:, 56:64], SIG)
                    tc0 = sp.tile([128, SB], F32, tag=f"tc0{h}")
                    nc.vector.tensor_mul(nt[:, SB : 2 * SB], gt[:, 24:32], gt[:, 8:16])
                    nc.scalar.activation(tc0[:], nt[:, SB : 2 * SB], TANH)
                    nc.vector.tensor_mul(nt[:, 3 * SB : 4 * SB], gt[:, 56:64], tc0[:])
                    mv = masuv[:, n : n + 1, 8 * h : 8 * h + 8].broadcast_to(
                        (128, 2, 8)
                    )
                    nc.vector.copy_predicated(
                        st.rearrange("p (r s b) -> p r s b", r=2, s=2)[:, :, 1, :],
                        mv,
                        nt.rearrange("p (r s b) -> p r s b", r=2, s=2)[:, :, 1, :],
                    )
                else:
                    # n == T: layer1 only
                    nc.scalar.activation(gt[:, 0:8], zps[:, 0:8], TANH)
                    nc.scalar.activation(gt[:, 16:24], zps[:, 16:24], SIG)
                    nc.scalar.activation(gt[:, 32:40], zps[:, 32:40], SIG)
                    nc.scalar.activation(gt[:, 48:56], zps[:, 48:56], SIG)
                    tmpa = sp.tile([128, SB], F32, tag=f"tmpa{h}")
                    tmpb = sp.tile([128, SB], F32, tag=f"tmpb{h}")
                    tc1 = sp.tile([128, SB], F32, tag=f"tc1{h}")
                    nc.vector.tensor_mul(tmpa[:], gt[:, 16:24], gt[:, 0:8])
                    nc.vector.tensor_mul(tmpb[:], gt[:, 32:40], st[:, 0:SB])
                    nc.vector.tensor_add(nt[:, 0:SB], tmpa[:], tmpb[:])
                    nc.scalar.activation(tc1[:], nt[:, 0:SB], TANH)
                    nc.vector.tensor_mul(nt[:, 2 * SB : 3 * SB], gt[:, 48:56], tc1[:])
                    mv = masuv[:, n - 1 : n, 8 * h : 8 * h + 8].broadcast_to(
                        (128, 2, 8)
                    )
                    nc.vector.copy_predicated(
                        st.rearrange("p (r s b) -> p r s b", r=2, s=2)[:, :, 0, :],
                        mv,
                        nt.rearrange("p (r s b) -> p r s b", r=2, s=2)[:, :, 0, :],
                    )

            def emit_send(n, h):
                # ---- ship AG(n) = {h1(n-1), h0(n)} ----
                # PE-transpose h to [16, 128] (feature-major rows), cast to
                # bf16, then a contiguous DMA feeds the AllGather.  Emitted
                # after BOTH halves' compute phases so the transpose's PE-FIFO
                # stall never blocks the other half's gate matmuls.
                st = state[h]
                cc_in = dp.tile([2 * SB, 128], BF16, tag=f"cc_in{h}")
                cc_out = dp.tile([16 * SB, 128], BF16, tag=f"cc_out{h}")
                tp = pp.tile([2 * SB, 128], F32, tag="tp", bufs=1)
                nc.tensor.transpose(tp[:], st[:, 2 * SB : 4 * SB], ident[:])
                hbt = sp.tile([2 * SB, 128], BF16, tag=f"hbt{h}")
                nc.scalar.activation(
                    hbt[:], tp[:], mybir.ActivationFunctionType.Copy
                )
                nc.scalar.dma_start(cc_in[:], hbt[:])
                nc.gpsimd.collective_compute(
                    "AllGather",
                    mybir.AluOpType.bypass,
                    ins=[cc_in.opt()],
                    outs=[cc_out.opt()],
                    replica_groups=rg,
                )
                cc_outs[n][h] = cc_out

            for n in range(T + 1):
                emit_substep(n, 0)
                for tcn, pn in proj_sched.get(n, []):
                    emit_proj(tcn, pn)
                    proj_done.add((tcn, pn))
                emit_substep(n, 1)
                for h in range(2):
                    emit_send(n, h)

            # ---- epilogue: receive AG(T), stage h1(T-1), finish proj ----
            for h in range(2):
                hfullT = receive(cc_outs[T][h], h)
                stage_h1(hfullT, T - 1, h)
            for tcn in range(NTC):
                for n in range(8):
                    if (tcn, n) not in proj_done:
                        emit_proj(tcn, n)
    return nc


_NC_CACHE = [None]


def kernel(tokens, emb, Wx0, Wh0, b0, Wx1, Wh1, b1, Wout, bout):
    tokens = np.asarray(tokens)
    toks = tokens.astype(np.int64)
    emb = np.asarray(emb, np.float32)
    fm = (toks != 0).astype(np.float32)[:, :T]  # [B,T]

    x = emb[toks]  # [B,T,E]
    xt = np.ascontiguousarray(x[:, :T].transpose(2, 1, 0).reshape(E, T * B))
    xt = xt.astype(ml_dtypes.bfloat16)  # cols (t, b)

    fm_tb = np.ascontiguousarray(fm.T).reshape(-1)  # (t,b) order
    masku = np.broadcast_to(fm_tb.reshape(1, T * B), (128, T * B)).astype(np.uint8)
    maskf = masku.astype(ml_dtypes.bfloat16)
    imptb = (1.0 - fm_tb).reshape(1, T * B).astype(ml_dtypes.bfloat16)

    GO = [2, 0, 1, 3]  # gate order [g,i,f,o] from original (i,f,g,o)

    def pack(w, nk):
        # w: [nk*128, 512 cols in gate order] -> [128, nk*4*128]
        a = np.asarray(w, np.float32).reshape(nk, 128, 4, 128)
        return (
            np.ascontiguousarray(a.transpose(1, 0, 2, 3))
            .reshape(128, nk * 4 * 128)
            .astype(ml_dtypes.bfloat16)
        )

    ones = np.ones((1, 128), ml_dtypes.bfloat16)
    bouta = np.asarray(bout, np.float32)
    onehot0 = np.zeros((V,), np.float32)
    onehot0[0] = 1.0

    in_maps = []
    for r in range(NC):
        sl = np.arange(128 * r, 128 * (r + 1))
        cols = np.concatenate([g * H + sl for g in GO])
        w0 = np.concatenate([np.asarray(Wx0)[:, cols], np.asarray(Wh0)[:, cols]], 0)
        w1 = np.concatenate([np.asarray(Wx1)[:, cols], np.asarray(Wh1)[:, cols]], 0)
        wo = np.asarray(Wout, np.float32)[:, VS * r : VS * (r + 1)]  # [1024, VS]
        woutp = (
            np.ascontiguousarray(wo.reshape(8, 128, VS).transpose(1, 0, 2))
            .reshape(128, 8 * VS)
            .astype(ml_dtypes.bfloat16)
        )
        b0a = np.asarray(b0, np.float32)
        b1a = np.asarray(b1, np.float32)
        # bias row groups [g1 g0 i1 i0 f1 f0 o1 o0], original gates (i,f,g,o)
        brow = np.concatenate(
            [
                b1a[2 * H + sl], b0a[2 * H + sl],
                b1a[0 * H + sl], b0a[0 * H + sl],
                b1a[1 * H + sl], b0a[1 * H + sl],
                b1a[3 * H + sl], b0a[3 * H + sl],
            ]
        ).reshape(1, 8 * 128).astype(ml_dtypes.bfloat16)
        vsl = slice(VS * r, VS * (r + 1))
        in_maps.append(
            {
                "w0p": pack(w0, 12),
                "w1p": pack(w1, 16),
                "woutp": woutp,
                "brow": brow,
                "onesr": ones,
                "boutr": bouta[vsl].reshape(1, VS).astype(ml_dtypes.bfloat16),
                "ohcorr": (onehot0[vsl] - bouta[vsl])
                .reshape(1, VS)
                .astype(ml_dtypes.bfloat16),
                "imptb": imptb,
                "xt": xt,
                "masku": masku,
                "maskf": maskf,
                "ident": np.eye(128, dtype=np.float32),
                "identb": np.eye(128, dtype=ml_dtypes.bfloat16),
            }
        )

    if _NC_CACHE[0] is None:
        _NC_CACHE[0] = build_nc()
    nc = _NC_CACHE[0]

    trace = os.environ.get("KERNEL_TRACE", "0") == "1"
    res = run_bass_kernel_spmd(
        nc, in_maps, core_ids=list(range(NC)), trace=trace
    )
    if trace and res.exec_time_ns is not None:
        print(f"HW exec time: {res.exec_time_ns} ns")

    logits = np.concatenate(
        [res.results[r]["out"] for r in range(NC)], axis=1
    )  # [(t,b), V]
    out = np.ascontiguousarray(
        logits.reshape(T, B, V).transpose(1, 0, 2)
    ).astype(np.float32)
    if T < tokens.shape[1]:
        full = np.zeros((B, tokens.shape[1], V), np.float32)
        full[:, :T] = out
        out = full
    return out


# revision 36
# speedup vs baseline: 1.1672x; 1.1005x over previous
"""Trainium2 Bass kernel for nn_LmLSTM: embedding -> 2x masked LSTM -> vocab projection.

Sharding: gate-sharded recurrence (core r owns hidden slice [128r,128r+128) of
both layers); full hidden state reassembled each step via AllGather of bf16
h-shards. The batch (B=16) is split into two halves that run as two
software-pipelined recurrences offset by half a step, so each half's
AllGather + sync latency hides behind the other half's compute.

The [H,V] output projection is vocab-sharded (4000 cols/core); tokens are laid
out (t, b)-major so projection chunks complete throughout the recurrence and
their GEMM work fills the PE idle time inside the recurrence loop. The whole
projection epilogue (bias, mask, onehot-for-masked-rows) is folded into the
PSUM accumulation via K=1 matmuls, and logits DMA straight from PSUM.

Per-step cell math for both layers is fused: one PSUM tile holds all 8 gate
groups [g1 g0 i1 i0 f1 f0 o1 o0] (biases pre-accumulated via K=1 matmuls), and
the elementwise chain runs on [128, 2*SB] combined tiles.
"""

import os
import sys
import types

import numpy as np
import ml_dtypes

# ---------------------------------------------------------------------------
# Environment shims (self-contained): NTFF profile hook + walrus wait-split.
# ---------------------------------------------------------------------------


def _install_axon_profile_hook():
    if "antenv.axon_hooks" in sys.modules:
        return
    holder = [None]
    mod = types.ModuleType("antenv.axon_hooks")
    mod.set_axon_ntff_profile_hook = lambda h: holder.__setitem__(0, h)
    mod.get_axon_ntff_profile_hook = lambda: holder[0]
    sys.modules["antenv.axon_hooks"] = mod
    try:
        import antenv

        antenv.axon_hooks = mod
        from trn_agent_boot.trn_boot import _ntff_profile_via_ctypes

        mod.set_axon_ntff_profile_hook(
            _ntff_profile_via_ctypes("/opt/axon/libaxon_pjrt.so")
        )
    except Exception:
        pass


_install_axon_profile_hook()

import concourse.bass as bass  # noqa: E402
import concourse.mybir as mybir  # noqa: E402
import concourse.tile as tile  # noqa: E402
from concourse.bass_utils import run_bass_kernel_spmd  # noqa: E402


def _install_wait_split():
    """This container's walrus accepts at most one sem-wait per instruction.
    Hoist excess waits onto same-engine nops placed just before."""
    if getattr(bass.Bass, "_waitsplit_installed", False):
        return
    counter = [0]

    def _split(m):
        for f in m.functions:
            for bb in f.blocks:
                il = bb.instructions
                if not any(
                    i.sync_info is not None and len(i.sync_info.on_wait) > 1
                    for i in il
                ):
                    continue
                new = []
                for inst in il:
                    si = inst.sync_info
                    if si is not None and len(si.on_wait) > 1:
                        waits = list(si.on_wait)
                        si.on_wait = waits[:1]
                        for w in waits[1:]:
                            counter[0] += 1
                            nop = mybir.InstNoOp(
                                name=f"waitsplit_{counter[0]}", ins=[], outs=[]
                            )
                            nop.engine = inst.engine
                            nop.sync_info = mybir.SyncInfo(
                                on_wait=[w], on_update=[]
                            )
                            new.append(nop)
                    new.append(inst)
                il.clear()
                il.extend(new)

    orig = bass.Bass.to_json_bytes

    def patched(self, *a, **kw):
        _split(self.m)
        return orig(self, *a, **kw)

    bass.Bass.to_json_bytes = patched
    bass.Bass._waitsplit_installed = True


_install_wait_split()

# ---------------------------------------------------------------------------
# Problem constants
# ---------------------------------------------------------------------------
V, E, H = 32000, 512, 1024
B = 16
T = int(os.environ.get("KERNEL_T", "256"))
NC = 8
VS = V // NC  # 4000 vocab cols per core
NTOK = B * T
NTC = NTOK // 128  # token chunks (8 t-steps x 16 b each)
SB = 16  # full batch width (single chain)
NSZ = VS // 8  # 500 vocab cols per projection n-group
F32 = mybir.dt.float32
BF16 = mybir.dt.bfloat16
U8 = mybir.dt.uint8
SIG = mybir.ActivationFunctionType.Sigmoid
TANH = mybir.ActivationFunctionType.Tanh

# psum gate-group layout (col = group*SB): [g1 g0 i1 i0 f1 f0 o1 o0]
G_G1, G_G0, G_I1, G_I0, G_F1, G_F0, G_O1, G_O0 = range(8)


def build_nc():
    nc = bass.Bass()
    d_w0 = nc.dram_tensor("w0p", [128, 12 * 4 * 128], BF16, kind="ExternalInput")
    d_w1 = nc.dram_tensor("w1p", [128, 16 * 4 * 128], BF16, kind="ExternalInput")
    d_wout = nc.dram_tensor("woutp", [128, 8 * VS], BF16, kind="ExternalInput")
    d_brow = nc.dram_tensor("brow", [1, 8 * 128], BF16, kind="ExternalInput")
    d_ones = nc.dram_tensor("onesr", [1, 128], BF16, kind="ExternalInput")
    d_boutr = nc.dram_tensor("boutr", [1, VS], BF16, kind="ExternalInput")
    d_ohcorr = nc.dram_tensor("ohcorr", [1, VS], BF16, kind="ExternalInput")
    d_imptb = nc.dram_tensor("imptb", [1, NTOK], BF16, kind="ExternalInput")
    d_xt = nc.dram_tensor("xt", [E, T * 16], BF16, kind="ExternalInput")
    d_masku = nc.dram_tensor("masku", [128, T * 16], U8, kind="ExternalInput")
    d_maskf = nc.dram_tensor("maskf", [128, T * 16], BF16, kind="ExternalInput")
    d_ident = nc.dram_tensor("ident", [128, 128], F32, kind="ExternalInput")
    d_identb = nc.dram_tensor("identb", [128, 128], BF16, kind="ExternalInput")
    d_out = nc.dram_tensor("out", [NTOK, VS], F32, kind="ExternalOutput")

    rg = [list(range(NC))]

    with tile.TileContext(nc) as tc:
        with (
            tc.tile_pool(name="wp", bufs=1) as wp,
            tc.tile_pool(name="sp", bufs=4) as sp,
            tc.tile_pool(name="pp", bufs=2, space="PSUM") as pp,
            tc.tile_pool(name="qq", bufs=2, space="PSUM") as qq,
            tc.tile_pool(name="dp", bufs=8, space="DRAM") as dp,
        ):
            # ---- persistent loads ----
            w0t = wp.tile([128, 12 * 4 * 128], BF16, tag="w0t")
            w1t = wp.tile([128, 16 * 4 * 128], BF16, tag="w1t")
            woutt = wp.tile([128, 8 * VS], BF16, tag="woutt")
            browt = wp.tile([1, 8 * 128], BF16, tag="browt")
            onest = wp.tile([1, 128], BF16, tag="onest")
            boutrt = wp.tile([1, VS], BF16, tag="boutrt")
            ohct = wp.tile([1, VS], BF16, tag="ohct")
            imptt = wp.tile([1, NTOK], BF16, tag="imptt")
            masku = wp.tile([128, T * 16], U8, tag="masku")
            maskf = wp.tile([128, T * 16], BF16, tag="maskf")
            ident = wp.tile([128, 128], F32, tag="ident")
            identb = wp.tile([128, 128], BF16, tag="identb")
            nc.gpsimd.dma_start(ident[:], d_ident[:])
            nc.gpsimd.dma_start(identb[:], d_identb[:])
            nc.gpsimd.dma_start(w0t[:], d_w0[:])
            nc.gpsimd.dma_start(w1t[:], d_w1[:])
            nc.gpsimd.dma_start(browt[:], d_brow[:])
            nc.gpsimd.dma_start(onest[:], d_ones[:])
            nc.gpsimd.dma_start(boutrt[:], d_boutr[:])
            nc.gpsimd.dma_start(ohct[:], d_ohcorr[:])
            nc.gpsimd.dma_start(imptt[:], d_imptb[:])
            nc.gpsimd.dma_start(masku[:], d_masku[:])
            nc.gpsimd.dma_start(maskf[:], d_maskf[:])
            nc.sync.dma_start(woutt[:], d_wout[:])
            xt = []
            for k in range(4):
                xk = wp.tile([128, T * 16], BF16, tag=f"xt{k}")
                nc.gpsimd.dma_start(xk[:], d_xt[128 * k : 128 * (k + 1), :])
                xt.append(xk)

            # stage ping-pong buffers: pre-masked h1 history for one proj
            # chunk, layout [128, (k 8)(j 8)(b 16)]
            stage0 = wp.tile([128, 8 * 8 * 16], BF16, tag="stage0")
            stage1 = wp.tile([128, 8 * 8 * 16], BF16, tag="stage1")
            stages = [stage0, stage1]

            # per-half persistent state: [c1 c0 h1 h0] f32
            state_a = wp.tile([128, 4 * SB], F32, tag="state_a")
            state = [state_a]
            nc.vector.memset(state_a[:], 0.0)

            masuv = masku.rearrange("p (t b) -> p t b", b=16)
            masfv = maskf.rearrange("p (t b) -> p t b", b=16)

            # ---------------- projection -------------------------------
            def emit_proj(tcn, n):
                ps = qq.tile([128, NSZ], F32, tag="proj", bufs=1)
                stg = stages[tcn % 2]
                # bias row: ones (x) bout_n
                nc.tensor.matmul(
                    ps[:],
                    onest[:1, :],
                    boutrt[:1, n * NSZ : (n + 1) * NSZ],
                    start=True,
                    stop=False,
                )
                # masked-row correction: impt_tc (x) (onehot0 - bout)_n
                nc.tensor.matmul(
                    ps[:],
                    imptt[:1, 128 * tcn : 128 * (tcn + 1)],
                    ohct[:1, n * NSZ : (n + 1) * NSZ],
                    start=False,
                    stop=False,
                )
                for k in range(8):
                    nc.tensor.matmul(
                        ps[:],
                        stg[:, 128 * k : 128 * (k + 1)],
                        woutt[:, k * VS + n * NSZ : k * VS + (n + 1) * NSZ],
                        start=False,
                        stop=(k == 7),
                    )
                lg = sp.tile([128, NSZ], F32, tag="lg")
                nc.vector.tensor_copy(lg[:], ps[:])
                nc.gpsimd.dma_start(
                    d_out[128 * tcn : 128 * (tcn + 1), n * NSZ : (n + 1) * NSZ],
                    lg[:],
                )

            # proj (tcn, n) emitted at iteration 8*tcn + 10 + n
            proj_sched = {}
            for tcn in range(NTC):
                for n in range(8):
                    proj_sched.setdefault(8 * tcn + 10 + n, []).append((tcn, n))
            proj_done = set()

            # ---------------- recurrence ---------------------------------
            # iteration n computes h0(n) (n<T) and h1(n-1) (n>=1), then ships
            # AG(n) = {h1(n-1), h0(n)}.  hfull from AG(n-1) holds blocks
            # [h1_k(n-2) at 2k, h0_k(n-1) at 2k+1].
            cc_outs = [[None] * 2 for _ in range(T + 1)]

            def stage_h1(hfull, j, h):
                stg = stages[(j // 8) % 2]
                src = hfull.rearrange("p (k l b) -> p k l b", k=8, l=2)[
                    :, :, 0, :
                ]
                dst = stg.rearrange("p (k j b) -> p k j b", k=8, j=8)[
                    :, :, j % 8, SB * h : SB * h + SB
                ]
                mkv = masfv[:, j : j + 1, SB * h : SB * h + SB].broadcast_to(
                    (128, 8, SB)
                )
                nc.vector.tensor_mul(dst, src, mkv)

            def receive(cco, h):
                # contiguous DMA of the AG output, then PE-transpose to land
                # it feature-major: hfull[p, k*16 + l*8 + b], l=0 h1, l=1 h0
                raw = sp.tile([128, 16 * SB], BF16, tag=f"raw{h}")
                nc.sync.dma_start(
                    raw.rearrange("p (j c) -> p j c", j=2),
                    cco.rearrange("(j p) c -> p j c", p=128),
                )
                pt = pp.tile([128, 16 * SB], BF16, tag=f"pt{h}", bufs=1)
                for j in range(2):
                    nc.tensor.transpose(
                        pt[:, 128 * j : 128 * (j + 1)],
                        raw[:, 128 * j : 128 * (j + 1)],
                        identb[:],
                    )
                hfull = sp.tile([128, 16 * SB], BF16, tag=f"hfull{h}")
                nc.vector.tensor_copy(hfull[:], pt[:])
                return hfull

            def emit_substep(n, h):
                # ---- receive AG(n-1) ----
                hfull = None
                if n >= 1:
                    hfull = receive(cc_outs[n - 1][h], h)
                    if n >= 2:
                        stage_h1(hfull, n - 2, h)

                # ---- gate matmuls into one PSUM tile ----
                zps = pp.tile([128, 8 * SB], F32, tag=f"z{h}")

                def gate_group(g, wtile, wbase, rhss):
                    dst = zps[:, g * SB : (g + 1) * SB]
                    nc.tensor.matmul(
                        dst,
                        browt[:, g * 128 : (g + 1) * 128],
                        onest[:1, 0:SB],
                        start=True,
                        stop=False,
                    )
                    nk = len(rhss)
                    for k, rhs in enumerate(rhss):
                        nc.tensor.matmul(
                            dst,
                            wtile[
                                :, (wbase + k * 4) * 128 : (wbase + k * 4) * 128 + 128
                            ],
                            rhs,
                            start=False,
                            stop=(k == nk - 1),
                        )

                l0 = n < T
                l1 = n >= 1
                if l1:
                    h0s = [hfull[:, 2 * SB * k + SB : 2 * SB * k + 2 * SB] for k in range(8)]
                    h1s = [hfull[:, 2 * SB * k : 2 * SB * k + SB] for k in range(8)]
                    rhs1 = h0s + h1s
                if l0:
                    rhs0 = [
                        xk[:, n * 16 : n * 16 + SB] for xk in xt
                    ] + (
                        h0s if n >= 1 else []
                    )
                # gate order: g first (starts the chain), then i, f, o
                # gi is the gate's index in the packed weight layout [g,i,f,o]
                for gname, gi in (("g", 0), ("i", 1), ("f", 2), ("o", 3)):
                    if l1:
                        g1 = {"g": G_G1, "i": G_I1, "f": G_F1, "o": G_O1}[gname]
                        gate_group(g1, w1t, gi, rhs1)
                    if l0 and not (n == 0 and gname == "f"):
                        g0 = {"g": G_G0, "i": G_I0, "f": G_F0, "o": G_O0}[gname]
                        gate_group(g0, w0t, gi, rhs0)

                # ---- fused cell math ----
                # state/nt layout: [c1 c0 h1 h0] (c block and h block each
                # contiguous [128, 2*SB] in layer order l1, l0)
                gt = sp.tile([128, 8 * SB], F32, tag=f"gt{h}")
                nt = sp.tile([128, 4 * SB], F32, tag=f"nt{h}")
                st = state[h]
                CB, HB = slice(0, 2 * SB), slice(2 * SB, 4 * SB)

                def lv(ap):  # view a [128, 2*SB] slice as [128, l=2, b]
                    return ap.rearrange("p (l b) -> p l b", l=2)

                S2, S4, S6, S8 = 2 * SB, 4 * SB, 6 * SB, 8 * SB
                if l0 and l1:
                    nc.scalar.activation(gt[:, 0:S2], zps[:, 0:S2], TANH)
                    nc.scalar.activation(gt[:, S2:S4], zps[:, S2:S4], SIG)
                    nc.scalar.activation(gt[:, S4:S6], zps[:, S4:S6], SIG)
                    nc.scalar.activation(gt[:, S6:S8], zps[:, S6:S8], SIG)
                    tmpa = sp.tile([128, 2 * SB], F32, tag=f"tmpa{h}")
                    tmpb = sp.tile([128, 2 * SB], F32, tag=f"tmpb{h}")
                    tcn_ = sp.tile([128, 2 * SB], F32, tag=f"tcn{h}")
                    nc.vector.tensor_mul(tmpa[:], gt[:, S2:S4], gt[:, 0:S2])
                    nc.vector.tensor_mul(tmpb[:], gt[:, S4:S6], st[:, CB])
                    nc.vector.tensor_add(nt[:, CB], tmpa[:], tmpb[:])
                    nc.scalar.activation(tcn_[:], nt[:, CB], TANH)
                    nc.vector.tensor_mul(nt[:, HB], gt[:, S6:S8], tcn_[:])
                    mv = (
                        masuv[:, n - 1 : n + 1, SB * h : SB * h + SB]
                        .unsqueeze(1)
                        .broadcast_to((128, 2, 2, SB))
                    )
                    nc.vector.copy_predicated(
                        st.rearrange("p (r s b) -> p r s b", r=2, s=2),
                        mv,
                        nt.rearrange("p (r s b) -> p r s b", r=2, s=2),
                    )
                elif l0:
                    # n == 0: layer0 only, c=0 so cn = i*g
                    nc.scalar.activation(gt[:, SB : 2 * SB], zps[:, SB : 2 * SB], TANH)
                    nc.scalar.activation(gt[:, 3 * SB : 4 * SB], zps[:, 3 * SB : 4 * SB], SIG)
                    nc.scalar.activation(gt[:, 7 * SB : 8 * SB], zps[:, 7 * SB : 8 * SB], SIG)
                    tc0 = sp.tile([128, SB], F32, tag=f"tc0{h}")
                    nc.vector.tensor_mul(
                        nt[:, SB : 2 * SB], gt[:, 3 * SB : 4 * SB], gt[:, SB : 2 * SB]
                    )
                    nc.scalar.activation(tc0[:], nt[:, SB : 2 * SB], TANH)
                    nc.vector.tensor_mul(
                        nt[:, 3 * SB : 4 * SB], gt[:, 7 * SB : 8 * SB], tc0[:]
                    )
                    mv = masuv[:, n : n + 1, SB * h : SB * h + SB].broadcast_to(
                        (128, 2, SB)
                    )
                    nc.vector.copy_predicated(
                        st.rearrange("p (r s b) -> p r s b", r=2, s=2)[:, :, 1, :],
                        mv,
                        nt.rearrange("p (r s b) -> p r s b", r=2, s=2)[:, :, 1, :],
                    )
                else:
                    # n == T: layer1 only
                    nc.scalar.activation(gt[:, 0:SB], zps[:, 0:SB], TANH)
                    nc.scalar.activation(gt[:, 2 * SB : 3 * SB], zps[:, 2 * SB : 3 * SB], SIG)
                    nc.scalar.activation(gt[:, 4 * SB : 5 * SB], zps[:, 4 * SB : 5 * SB], SIG)
                    nc.scalar.activation(gt[:, 6 * SB : 7 * SB], zps[:, 6 * SB : 7 * SB], SIG)
                    tmpa = sp.tile([128, SB], F32, tag=f"tmpa{h}")
                    tmpb = sp.tile([128, SB], F32, tag=f"tmpb{h}")
                    tc1 = sp.tile([128, SB], F32, tag=f"tc1{h}")
                    nc.vector.tensor_mul(tmpa[:], gt[:, 2 * SB : 3 * SB], gt[:, 0:SB])
                    nc.vector.tensor_mul(tmpb[:], gt[:, 4 * SB : 5 * SB], st[:, 0:SB])
                    nc.vector.tensor_add(nt[:, 0:SB], tmpa[:], tmpb[:])
                    nc.scalar.activation(tc1[:], nt[:, 0:SB], TANH)
                    nc.vector.tensor_mul(
                        nt[:, 2 * SB : 3 * SB], gt[:, 6 * SB : 7 * SB], tc1[:]
                    )
                    mv = masuv[:, n - 1 : n, SB * h : SB * h + SB].broadcast_to(
                        (128, 2, SB)
                    )
                    nc.vector.copy_predicated(
                        st.rearrange("p (r s b) -> p r s b", r=2, s=2)[:, :, 0, :],
                        mv,
                        nt.rearrange("p (r s b) -> p r s b", r=2, s=2)[:, :, 0, :],
                    )

            def emit_send(n, h):
                # ---- ship AG(n) = {h1(n-1), h0(n)} ----
                # PE-transpose h to [16, 128] (feature-major rows), cast to
                # bf16, then a contiguous DMA feeds the AllGather.  Emitted
                # after BOTH halves' compute phases so the transpose's PE-FIFO
                # stall never blocks the other half's gate matmuls.
                st = state[h]
                cc_in = dp.tile([2 * SB, 128], BF16, tag=f"cc_in{h}")
                cc_out = dp.tile([16 * SB, 128], BF16, tag=f"cc_out{h}")
                tp = pp.tile([2 * SB, 128], F32, tag="tp", bufs=1)
                nc.tensor.transpose(tp[:], st[:, 2 * SB : 4 * SB], ident[:])
                hbt = sp.tile([2 * SB, 128], BF16, tag=f"hbt{h}")
                nc.scalar.activation(
                    hbt[:], tp[:], mybir.ActivationFunctionType.Copy
                )
                nc.scalar.dma_start(cc_in[:], hbt[:])
                nc.gpsimd.collective_compute(
                    "AllGather",
                    mybir.AluOpType.bypass,
                    ins=[cc_in.opt()],
                    outs=[cc_out.opt()],
                    replica_groups=rg,
                )
                cc_outs[n][h] = cc_out

            for n in range(T + 1):
                emit_substep(n, 0)
                for tcn, pn in proj_sched.get(n, []):
                    emit_proj(tcn, pn)
                    proj_done.add((tcn, pn))
                emit_send(n, 0)

            # ---- epilogue: receive AG(T), stage h1(T-1), finish proj ----
            hfullT = receive(cc_outs[T][0], 0)
            stage_h1(hfullT, T - 1, 0)
            for tcn in range(NTC):
                for n in range(8):
                    if (tcn, n) not in proj_done:
                        emit_proj(tcn, n)
    return nc


_NC_CACHE = [None]


def kernel(tokens, emb, Wx0, Wh0, b0, Wx1, Wh1, b1, Wout, bout):
    tokens = np.asarray(tokens)
    toks = tokens.astype(np.int64)
    emb = np.asarray(emb, np.float32)
    fm = (toks != 0).astype(np.float32)[:, :T]  # [B,T]

    x = emb[toks]  # [B,T,E]
    xt = np.ascontiguousarray(x[:, :T].transpose(2, 1, 0).reshape(E, T * B))
    xt = xt.astype(ml_dtypes.bfloat16)  # cols (t, b)

    fm_tb = np.ascontiguousarray(fm.T).reshape(-1)  # (t,b) order
    masku = np.broadcast_to(fm_tb.reshape(1, T * B), (128, T * B)).astype(np.uint8)
    maskf = masku.astype(ml_dtypes.bfloat16)
    imptb = (1.0 - fm_tb).reshape(1, T * B).astype(ml_dtypes.bfloat16)

    GO = [2, 0, 1, 3]  # gate order [g,i,f,o] from original (i,f,g,o)

    def pack(w, nk):
        # w: [nk*128, 512 cols in gate order] -> [128, nk*4*128]
        a = np.asarray(w, np.float32).reshape(nk, 128, 4, 128)
        return (
            np.ascontiguousarray(a.transpose(1, 0, 2, 3))
            .reshape(128, nk * 4 * 128)
            .astype(ml_dtypes.bfloat16)
        )

    ones = np.ones((1, 128), ml_dtypes.bfloat16)
    bouta = np.asarray(bout, np.float32)
    onehot0 = np.zeros((V,), np.float32)
    onehot0[0] = 1.0

    in_maps = []
    for r in range(NC):
        sl = np.arange(128 * r, 128 * (r + 1))
        cols = np.concatenate([g * H + sl for g in GO])
        w0 = np.concatenate([np.asarray(Wx0)[:, cols], np.asarray(Wh0)[:, cols]], 0)
        w1 = np.concatenate([np.asarray(Wx1)[:, cols], np.asarray(Wh1)[:, cols]], 0)
        wo = np.asarray(Wout, np.float32)[:, VS * r : VS * (r + 1)]  # [1024, VS]
        woutp = (
            np.ascontiguousarray(wo.reshape(8, 128, VS).transpose(1, 0, 2))
            .reshape(128, 8 * VS)
            .astype(ml_dtypes.bfloat16)
        )
        b0a = np.asarray(b0, np.float32)
        b1a = np.asarray(b1, np.float32)
        # bias row groups [g1 g0 i1 i0 f1 f0 o1 o0], original gates (i,f,g,o)
        brow = np.concatenate(
            [
                b1a[2 * H + sl], b0a[2 * H + sl],
                b1a[0 * H + sl], b0a[0 * H + sl],
                b1a[1 * H + sl], b0a[1 * H + sl],
                b1a[3 * H + sl], b0a[3 * H + sl],
            ]
        ).reshape(1, 8 * 128).astype(ml_dtypes.bfloat16)
        vsl = slice(VS * r, VS * (r + 1))
        in_maps.append(
            {
                "w0p": pack(w0, 12),
                "w1p": pack(w1, 16),
                "woutp": woutp,
                "brow": brow,
                "onesr": ones,
                "boutr": bouta[vsl].reshape(1, VS).astype(ml_dtypes.bfloat16),
                "ohcorr": (onehot0[vsl] - bouta[vsl])
                .reshape(1, VS)
                .astype(ml_dtypes.bfloat16),
                "imptb": imptb,
                "xt": xt,
                "masku": masku,
                "maskf": maskf,
                "ident": np.eye(128, dtype=np.float32),
                "identb": np.eye(128, dtype=ml_dtypes.bfloat16),
            }
        )

    if _NC_CACHE[0] is None:
        _NC_CACHE[0] = build_nc()
    nc = _NC_CACHE[0]

    trace = os.environ.get("KERNEL_TRACE", "0") == "1"
    res = run_bass_kernel_spmd(
        nc, in_maps, core_ids=list(range(NC)), trace=trace
    )
    if trace and res.exec_time_ns is not None:
        print(f"HW exec time: {res.exec_time_ns} ns")

    logits = np.concatenate(
        [res.results[r]["out"] for r in range(NC)], axis=1
    )  # [(t,b), V]
    out = np.ascontiguousarray(
        logits.reshape(T, B, V).transpose(1, 0, 2)
    ).astype(np.float32)
    if T < tokens.shape[1]:
        full = np.zeros((B, tokens.shape[1], V), np.float32)
        full[:, :T] = out
        out = full
    return out
